# revision 38
# baseline (speedup 1.0000x reference)
"""Trainium2 Bass kernel for nn_CaptionDecoder (attention LSTM caption decoder).

Strategy (8 NeuronCores):
  Phase A: data-parallel over batch (8 batches/core) for the sequential
           attention + 2-layer-LSTM recurrence. Every per-step matmul is in
           transposed orientation (feature dim on PSUM partitions, batch as
           the streamed free dim), so gates, dec_proj, attention scores and
           context all come out of PSUM already transposed and the LSTM
           pointwise runs on 128-partition tiles; there are no per-step DMAs
           and no per-step PE transposes. The 8 batches are split into two
           independent 4-batch streams, software-pipelined with a half-step
           skew so one stream's attention tanh (the scalar-engine floor)
           overlaps the other stream's matmul/vector segments.
  Host:    gathers top-layer hidden states hb from the 8 cores (1.3 MB).
  Phase B: vocab-parallel logits projection in transposed layout
           (vocab tile on partitions, all 1280 (b,t) rows streamed), so
           b_out becomes a per-partition bias folded into the PSUM
           evacuation, which alternates between ACT and DVE to stay off
           the tensor-engine critical path.

Precision: bf16 matmuls with fp32 PSUM accumulation; fp32 cell state.
Sigmoid is computed as 0.5*(1+tanh(x/2)); the 0.5 factors are folded into
host-prescaled weights (column scale on f,i,o gates) and doubled states
H2=2h, C2=2c (row scale on recurrent weights), so one tanh over all 2048
gate pre-activations covers every gate nonlinearity.
"""

import numpy as np
import ml_dtypes

import concourse.bass as bass
import concourse.bacc as bacc
import concourse.mybir as mybir
import concourse.tile as tile
from concourse.bass import ts
from concourse.bass_utils import run_bass_kernel_spmd
from concourse.masks import make_identity

F32 = mybir.dt.float32
BF16 = mybir.dt.bfloat16
AF = mybir.ActivationFunctionType
ALU = mybir.AluOpType

B, TC, P, E, H, A, V = 64, 21, 196, 512, 512, 512, 30000
T = TC - 1            # 20 decode steps
NC = 8                # cores
BL = B // NC          # 8 batches per core
PPAD = 256            # padded attention positions per batch
NF = BL * PPAD // 128  # 16 (batch, p-half) chunks
BP = BL * P           # 1568 (p, b) columns per core
G4 = 4 * H            # 2048 stacked gates f,i,C,o
VSL = V // NC         # 3750 vocab rows per core (phase B)
BT = B * T            # 1280 (b, t) columns
P1 = P - 128          # 68 positions in the second p-half


def _bf16(x):
    return np.ascontiguousarray(np.asarray(x, dtype=np.float32)).astype(
        ml_dtypes.bfloat16)


def _sub(ap, dims, extra_offset=0):
    """Custom free-dim access pattern on an AP, keeping its partition dim."""
    return bass.AP(ap.tensor, ap.offset + extra_offset,
                   [list(ap.ap[0])] + [list(d) for d in dims])


def _pbcast(ap, dims, extra_offset=0):
    """Partition-broadcast (stride 0) custom AP."""
    return bass.AP(ap.tensor, ap.offset + extra_offset,
                   [[0, 128]] + [list(d) for d in dims])


# --------------------------------------------------------------------------
# Phase A module: the recurrence
# --------------------------------------------------------------------------

DBG = False


def build_phase_a(n_steps=T):
    nc = bacc.Bacc("TRN2", num_devices=NC, debug=False)

    def din(name, shape, dt=BF16):
        return nc.dram_tensor(name, shape, dt, kind="ExternalInput").ap()

    encT = din("encT", [4, 128, BP])          # encoder_out^T  [a-chk][a][(p,b)]
    encflat = din("encflat", [NF, 128, E])    # [(b,half)][p][e], 0-padded
    featT = din("featT", [4, 128, BL])
    wih2 = din("wih2", [4, 128, H])           # 2*W_ih
    wic2 = din("wic2", [4, 128, H])           # 2*W_ic
    bih2 = din("bih2", [1, H])
    bic2 = din("bic2", [1, H])
    wenc = din("wenc", [4, 128, A])
    biasadT = din("biasadT", [4, 128, 1])     # b_enc + b_dec, transposed
    wdech = din("wdech", [4, 128, A])         # 0.5*W_dec
    vcol = din("vcol", [4, 128, 1])
    weT = din("weT", [4, 128, T * BL])        # embeds^T, col = t*8+b
    wg0x = din("wg0x", [4, 128, G4])          # col-scaled
    bg0 = din("bg0", [1, G4])
    wg0c = din("wg0c", [4, 128, G4])
    wg0h = din("wg0h", [4, 128, G4])
    wg1a = din("wg1a", [4, 128, G4])
    wg1b = din("wg1b", [4, 128, G4])
    bg1 = din("bg1", [1, G4])

    hballT = nc.dram_tensor("hballT", [4, 128, T * BL], BF16,
                            kind="ExternalOutput").ap()
    if DBG:
        dbg = {
            "dbg_h0": nc.dram_tensor("dbg_h0", [128, 32], BF16,
                                     kind="ExternalOutput").ap(),
            "dbg_c0": nc.dram_tensor("dbg_c0", [128, 32], F32,
                                     kind="ExternalOutput").ap(),
            "dbg_encproj": nc.dram_tensor("dbg_encproj", [128, 4 * BP], BF16,
                                          kind="ExternalOutput").ap(),
            "dbg_decT": nc.dram_tensor("dbg_decT", [128, 32], BF16,
                                       kind="ExternalOutput").ap(),
            "dbg_ssb": nc.dram_tensor("dbg_ssb", [128, 4 * BP], BF16,
                                      kind="ExternalOutput").ap(),
            "dbg_aB": nc.dram_tensor("dbg_aB", [128, 16], BF16,
                                     kind="ExternalOutput").ap(),
            "dbg_ctxT": nc.dram_tensor("dbg_ctxT", [128, 32], BF16,
                                       kind="ExternalOutput").ap(),
            "dbg_tg0": nc.dram_tensor("dbg_tg0", [128, 128], BF16,
                                      kind="ExternalOutput").ap(),
            "dbg_wep": nc.dram_tensor("dbg_wep", [128, 16 * T * BL], BF16,
                                      kind="ExternalOutput").ap(),
        }

    with tile.TileContext(nc) as tc:
        with (
            tc.tile_pool(name="persist", bufs=1) as pp,
            tc.tile_pool(name="lp", bufs=2) as lp,
            tc.tile_pool(name="pa", bufs=1, space="PSUM") as pa,
            tc.tile_pool(name="pg", bufs=1, space="PSUM") as pg,
        ):
            def dma3(dst, src, n=4):  # dram [n,128,X] -> sbuf [128,n,X]
                for k in range(n):
                    nc.sync.dma_start(out=dst[:, k], in_=src[k])

            # ---- persistent weights / constants --------------------------
            sb_wdech = pp.tile([128, 4, A], BF16, tag="wdech")
            dma3(sb_wdech, wdech)
            sb_vcol = pp.tile([128, 4, 1], BF16, tag="vcol")
            dma3(sb_vcol, vcol)
            sb_biasadT = pp.tile([128, 4, 1], BF16, tag="biasadT")
            dma3(sb_biasadT, biasadT)
            sb_wg0c = pp.tile([128, 4, G4], BF16, tag="wg0c")
            dma3(sb_wg0c, wg0c)
            sb_wg0h = pp.tile([128, 4, G4], BF16, tag="wg0h")
            dma3(sb_wg0h, wg0h)
            sb_wg1a = pp.tile([128, 4, G4], BF16, tag="wg1a")
            dma3(sb_wg1a, wg1a)
            sb_wg1b = pp.tile([128, 4, G4], BF16, tag="wg1b")
            dma3(sb_wg1b, wg1b)
            sb_bg1 = pp.tile([1, G4], BF16, tag="bg1")
            nc.sync.dma_start(out=sb_bg1[:], in_=bg1)
            sb_encflat = pp.tile([128, NF, E], BF16, tag="encflat")
            dma3(sb_encflat, encflat, n=NF)

            i128 = pp.tile([128, 128], BF16, tag="i128")
            make_identity(nc, i128[:])
            ones_1x8 = pp.tile([1, 8], BF16, tag="o18")
            nc.vector.memset(ones_1x8[:], 1.0)
            ones_1xT8 = pp.tile([1, T * BL], BF16, tag="o1T8")
            nc.vector.memset(ones_1xT8[:], 1.0)
            ones_col = pp.tile([128, 1], BF16, tag="ocol")
            nc.vector.memset(ones_col[:], 1.0)
            ones_1x128f = pp.tile([1, 128], F32, tag="o1128f")
            nc.vector.memset(ones_1x128f[:], 1.0)

            # persistent state, two independent 4-batch streams
            # (all transposed: [128, 4, 4] = [dim%128, dim//128, batch-in-stream])
            BS = BL // 2
            C2aT = [pp.tile([128, 4, BS], F32, tag=f"C2aT{s}", name=f"C2aT{s}")
                    for s in (0, 1)]
            C2bT = [pp.tile([128, 4, BS], F32, tag=f"C2bT{s}", name=f"C2bT{s}")
                    for s in (0, 1)]
            H2aT = [pp.tile([128, 4, BS], BF16, tag=f"H2aT{s}", name=f"H2aT{s}")
                    for s in (0, 1)]
            H2b0 = [pp.tile([128, 4, BS], BF16, tag=f"H2b0{s}", name=f"H2b0{s}")
                    for s in (0, 1)]
            hball = pp.tile([128, 4, T * BL], BF16, tag="hball")

            aB = [pp.tile([128, 2 * BS], BF16, tag=f"aB{s}", name=f"aB{s}")
                  for s in (0, 1)]
            nc.vector.memset(aB[0][:], 0.0)
            nc.vector.memset(aB[1][:], 0.0)

            encproj = pp.tile([128, 4, BP], BF16, tag="encproj")
            wepartT = pp.tile([128, 16, T * BL], BF16, tag="wepartT")

            # ---- one-time section (own pool, freed before the loop) ------
            sp = tc.alloc_tile_pool(name="stream", bufs=1)
            sq = tc.alloc_tile_pool(name="sq", bufs=2, space="PSUM")
            sb_featT = sp.tile([128, 4, BL], BF16, tag="featT")
            dma3(sb_featT, featT)
            sb_wih2 = sp.tile([128, 4, H], BF16, tag="wih2")
            dma3(sb_wih2, wih2)
            sb_wic2 = sp.tile([128, 4, H], BF16, tag="wic2")
            dma3(sb_wic2, wic2)
            sb_bih2 = sp.tile([1, H], BF16, tag="bih2")
            nc.sync.dma_start(out=sb_bih2[:], in_=bih2)
            sb_bic2 = sp.tile([1, H], BF16, tag="bic2")
            nc.sync.dma_start(out=sb_bic2[:], in_=bic2)

            # h0/c0 transposed: out[h%128, hc, b]
            hc0 = sq.tile([128, 512], F32, tag="ot", name="hc0")
            for (w_sb, b_sb, co) in ((sb_wih2, sb_bih2, 0),
                                     (sb_wic2, sb_bic2, 32)):
                for hc in range(4):
                    reg = hc0[:, co + hc * 8:co + (hc + 1) * 8]
                    for k in range(4):
                        nc.tensor.matmul(reg, w_sb[:, k, ts(hc, 128)],
                                         sb_featT[:, k],
                                         start=(k == 0), stop=False,
                                         skip_group_check=True)
                    nc.tensor.matmul(reg, b_sb[:, ts(hc, 128)], ones_1x8[:],
                                     start=False, stop=True,
                                     skip_group_check=True)
            for s in (0, 1):
                for k in range(4):
                    h_sl = hc0[:, k * 8 + s * BS:k * 8 + (s + 1) * BS]
                    c_sl = hc0[:, 32 + k * 8 + s * BS:32 + k * 8 + (s + 1) * BS]
                    nc.scalar.activation(out=H2aT[s][:, k], in_=h_sl,
                                         func=AF.Copy)
                    nc.scalar.activation(out=H2b0[s][:, k], in_=h_sl,
                                         func=AF.Copy)
                    nc.vector.tensor_copy(C2aT[s][:, k], c_sl)
                    nc.vector.tensor_copy(C2bT[s][:, k], c_sl)

            # ---- one-time: enc_projT (+ bias) ----------------------------
            sb_encT = sp.tile([128, 4, BP], BF16, tag="encT")
            dma3(sb_encT, encT)
            sb_wenc = sp.tile([128, 4, A], BF16, tag="wenc")
            dma3(sb_wenc, wenc)
            QS = BP // 4  # 392
            for k in range(4):          # a-chunk
                for q in range(4):      # (p,b) quarter
                    eps = sq.tile([128, 512], F32, tag="ot", name="eps")
                    for e in range(4):  # e-chunk
                        nc.tensor.matmul(
                            eps[:, 0:QS], sb_wenc[:, e, ts(k, 128)],
                            sb_encT[:, e, ts(q, QS)],
                            start=(e == 0), stop=(e == 3))
                    nc.scalar.activation(out=encproj[:, k, ts(q, QS)],
                                         in_=eps[:, 0:QS], func=AF.Identity,
                                         bias=sb_biasadT[:, k])


            # ================= the recurrent steps ========================
            # Two 4-batch streams per core; emission ping-pongs between them
            # so one stream's big ACT tanh overlaps the other's DVE/PE work.
            def pointwise(nc, lp, gps, s, C2T, h2out, tag):
                """gps = strided [128, 16, BS] view of the gates psum."""
                tg = lp.tile([128, 16, BS], BF16, tag="tg" + tag)
                nc.scalar.activation(
                    out=tg[:].rearrange("p g b -> p (g b)"), in_=gps,
                    func=AF.Tanh)
                tf = _sub(tg[:], [[BS, 4], [1, BS]], extra_offset=0)
                ti = _sub(tg[:], [[BS, 4], [1, BS]], extra_offset=4 * BS)
                tC = _sub(tg[:], [[BS, 4], [1, BS]], extra_offset=8 * BS)
                to = _sub(tg[:], [[BS, 4], [1, BS]], extra_offset=12 * BS)
                s1 = lp.tile([128, 4, BS], F32, tag="s1" + tag)
                nc.vector.scalar_tensor_tensor(
                    out=s1[:], in0=tf, scalar=1.0, in1=C2T[:],
                    op0=ALU.add, op1=ALU.mult)
                s2 = lp.tile([128, 4, BS], F32, tag="s2" + tag)
                nc.vector.scalar_tensor_tensor(
                    out=s2[:], in0=ti, scalar=1.0, in1=tC,
                    op0=ALU.add, op1=ALU.mult)
                nc.vector.scalar_tensor_tensor(
                    out=C2T[:], in0=s1[:], scalar=0.5, in1=s2[:],
                    op0=ALU.mult, op1=ALU.add)
                tch = lp.tile([128, 4, BS], BF16, tag="tch" + tag)
                nc.scalar.activation(out=tch[:], in_=C2T[:],
                                     func=AF.Tanh, scale=0.5)
                nc.vector.scalar_tensor_tensor(
                    out=h2out, in0=to, scalar=1.0, in1=tch[:],
                    op0=ALU.add, op1=ALU.mult)

            # ---- one-time: wepartT = (we @ Wg0x + bg0)^T -----------------
            sb_weT = sp.tile([128, 4, T * BL], BF16, tag="weT")
            dma3(sb_weT, weT)
            sb_wg0x = sp.tile([128, 4, G4], BF16, tag="wg0x")
            dma3(sb_wg0x, wg0x)
            sb_bg0 = sp.tile([1, G4], BF16, tag="bg0")
            nc.sync.dma_start(out=sb_bg0[:], in_=bg0)
            for gc in range(16):
                wps = sq.tile([128, 512], F32, tag="ot", name="wps")
                nc.tensor.matmul(wps[:, 0:T * BL], sb_bg0[:, ts(gc, 128)],
                                 ones_1xT8[:], start=True, stop=False)
                for k in range(4):
                    nc.tensor.matmul(wps[:, 0:T * BL],
                                     sb_wg0x[:, k, ts(gc, 128)],
                                     sb_weT[:, k],
                                     start=False, stop=(k == 3))
                nc.vector.tensor_copy(wepartT[:, gc], wps[:, 0:T * BL])

            sp.release()
            sq.release()

            # ---- per-op emitters; state dicts keyed by stream ------------
            ST = [{"ps": None, "decT": None, "ssb": None, "rinvr": None,
                   "ctxT": None, "g0": None, "g1": None} for _ in (0, 1)]

            def H2bTk(s, k, t):
                if t == 0:
                    return H2b0[s][:, k]
                return _sub(hball[:], [[1, BS]],
                            extra_offset=k * T * BL + (t - 1) * BL + s * BS)

            def e_dec(s, t):
                st = ST[s]
                st["ps"] = pa.tile([128, 128], F32, tag=f"ps{s}",
                                   name=f"ps{s}")
                for ac in range(4):
                    reg = st["ps"][:, ac * BS:(ac + 1) * BS]
                    for k in range(4):
                        nc.tensor.matmul(reg, sb_wdech[:, k, ts(ac, 128)],
                                         H2bTk(s, k, t),
                                         start=(k == 0), stop=(k == 3),
                                         skip_group_check=True)

            def e_decT(s, k):
                st = ST[s]
                if k != 0:
                    return
                st["decT"] = lp.tile([128, 4, BS], BF16, tag=f"decT{s}",
                                     name=f"decT{s}")
                st["ssb"] = lp.tile([128, 4, P * BS], BF16,
                                    tag=f"ssb{s}", name=f"ssb{s}")
                nc.scalar.activation(
                    out=st["decT"][:].rearrange("p a b -> p (a b)"),
                    in_=st["ps"][:, 0:16], func=AF.Copy)

            def e_add(s, k):
                st = ST[s]
                nc.vector.tensor_tensor(
                    st["ssb"][:, k].rearrange("p (q b) -> p q b", b=BS),
                    _sub(encproj[:, k], [[BL, P], [1, BS]],
                         extra_offset=s * BS),
                    _sub(st["decT"][:], [[0, P], [1, BS]],
                         extra_offset=k * BS),
                    ALU.add)

            def e_tanh(s, k):
                nc.scalar.activation(out=ST[s]["ssb"][:, k],
                                     in_=ST[s]["ssb"][:, k], func=AF.Tanh)

            def e_scores(s):
                st = ST[s]
                for half, pw, poff in ((0, 128, 0), (1, P1, 128 * BS)):
                    for j in range(BS):
                        col = 32 + half * BS + j
                        for k in range(4):
                            nc.tensor.matmul(
                                st["ps"][:pw, col:col + 1],
                                _sub(st["ssb"][:, k], [[BS, pw]],
                                     extra_offset=poff + j),
                                sb_vcol[:, k],
                                start=(k == 0), stop=(k == 3),
                                skip_group_check=True)

            def e_exp(s):
                pst = ST[s]["ps"]
                nc.scalar.activation(out=aB[s][:, 0:BS],
                                     in_=pst[:, 32:32 + BS], func=AF.Exp)
                nc.scalar.activation(out=aB[s][0:P1, BS:2 * BS],
                                     in_=pst[0:P1, 32 + BS:32 + 2 * BS],
                                     func=AF.Exp)

            def e_sums(s):
                pst = ST[s]["ps"]
                nc.tensor.matmul(pst[0:1, 48:48 + BS], ones_col[:],
                                 aB[s][:, 0:BS], start=True, stop=False,
                                 skip_group_check=True)
                nc.tensor.matmul(pst[0:1, 48:48 + BS], ones_col[:],
                                 aB[s][:, BS:2 * BS], start=False,
                                 stop=True, skip_group_check=True)

            def e_recip(s):
                st = ST[s]
                st["rinvr"] = lp.tile([1, BS], F32, tag=f"rinvr{s}",
                                      name=f"rinvr{s}")
                nc.vector.reciprocal(st["rinvr"][:],
                                     st["ps"][0:1, 48:48 + BS])

            def e_ctx(s):
                pst = ST[s]["ps"]
                for j in range(BS):
                    b = s * BS + j
                    for ec in range(4):
                        col = 64 + ec * BS + j
                        for half in (0, 1):
                            nc.tensor.matmul(
                                pst[:, col:col + 1],
                                sb_encflat[:, 2 * b + half, ts(ec, 128)],
                                aB[s][:, half * BS + j:half * BS + j + 1],
                                start=(half == 0), stop=(half == 1),
                                skip_group_check=True)

            def e_ctxT(s):
                st = ST[s]
                rb = lp.tile([128, BS], F32, tag=f"rb{s}", name=f"rb{s}")
                nc.gpsimd.partition_broadcast(rb[:], st["rinvr"][:])
                st["ctxT"] = lp.tile([128, 4, BS], BF16, tag=f"ctxT{s}",
                                     name=f"ctxT{s}")
                nc.vector.tensor_tensor(
                    st["ctxT"][:],
                    _sub(st["ps"][:], [[BS, 4], [1, BS]], extra_offset=64),
                    _sub(rb[:], [[0, 4], [1, BS]]),
                    ALU.mult)

            def e_g0(s, t):
                st = ST[s]
                st["g0"] = pg.tile([128, 64], F32, tag=f"g0{s}",
                                   name=f"g0{s}")
                for gc in range(16):
                    reg = st["g0"][:, gc * BS:(gc + 1) * BS]
                    nc.tensor.matmul(
                        reg, i128[:],
                        wepartT[:, gc,
                                t * BL + s * BS:t * BL + (s + 1) * BS],
                        start=True, stop=False)
                    for k in range(4):
                        nc.tensor.matmul(reg, sb_wg0h[:, k, ts(gc, 128)],
                                         H2aT[s][:, k], start=False,
                                         stop=False)
                    for k in range(4):
                        nc.tensor.matmul(reg, sb_wg0c[:, k, ts(gc, 128)],
                                         st["ctxT"][:, k], start=False,
                                         stop=(k == 3))

            def e_pw0(s):
                pointwise(nc, lp, _sub(ST[s]["g0"][:], [[BS, 16], [1, BS]]),
                          s, C2aT[s], H2aT[s][:], f"0{s}")

            def e_g1(s, t):
                st = ST[s]
                st["g1"] = pg.tile([128, 64], F32, tag=f"g1{s}",
                                   name=f"g1{s}")
                for gc in range(16):
                    reg = st["g1"][:, gc * BS:(gc + 1) * BS]
                    nc.tensor.matmul(reg, sb_bg1[:, ts(gc, 128)],
                                     ones_1x8[:, 0:BS],
                                     start=True, stop=False)
                    for k in range(4):
                        nc.tensor.matmul(reg, sb_wg1b[:, k, ts(gc, 128)],
                                         H2bTk(s, k, t), start=False,
                                         stop=False)
                    for k in range(4):
                        nc.tensor.matmul(reg, sb_wg1a[:, k, ts(gc, 128)],
                                         H2aT[s][:, k], start=False,
                                         stop=(k == 3))

            def e_pw1(s, t):
                h2b_out = _sub(hball[:], [[T * BL, 4], [1, BS]],
                               extra_offset=t * BL + s * BS)
                pointwise(nc, lp, _sub(ST[s]["g1"][:], [[BS, 16], [1, BS]]),
                          s, C2bT[s], h2b_out, f"1{s}")

            def front(s, t):
                """dec + adds + tanh + scores, self-contained (prologue)."""
                e_dec(s, t)
                for k in range(4):
                    e_decT(s, k)
                    e_add(s, k)
                    e_tanh(s, k)
                e_scores(s)

            def slot(sb_, sf, t_back, t_front, do_front):
                """Emit stream sb_'s back-half of step t_back, micro-
                interleaved with stream sf's front-half of step t_front."""
                if do_front:
                    e_dec(sf, t_front)
                e_exp(sb_)
                if do_front:
                    e_decT(sf, 0)
                    e_add(sf, 0)
                e_sums(sb_)
                e_recip(sb_)
                if do_front:
                    e_tanh(sf, 0)
                    e_decT(sf, 1)
                    e_add(sf, 1)
                e_ctx(sb_)
                e_ctxT(sb_)
                if do_front:
                    e_tanh(sf, 1)
                    e_decT(sf, 2)
                e_g0(sb_, t_back)
                if do_front:
                    e_add(sf, 2)
                e_pw0(sb_)
                if do_front:
                    e_tanh(sf, 2)
                    e_decT(sf, 3)
                    e_add(sf, 3)
                e_g1(sb_, t_back)
                if do_front:
                    e_tanh(sf, 3)
                e_pw1(sb_, t_back)
                if do_front:
                    e_scores(sf)

            # ---- software-pipelined schedule: stream 1 lags half a step --
            front(0, 0)
            for t in range(n_steps):
                slot(0, 1, t, t, True)                       # s0 back | s1 front
                slot(1, 0, t, t + 1, t + 1 < n_steps)        # s1 back | s0 front

            for k in range(4):
                nc.sync.dma_start(out=hballT[k], in_=hball[:, k])

    nc.compile()
    return nc


# --------------------------------------------------------------------------
# Phase B module: logitsT = (0.5*W_out)^T @ H2b_all + b_out  (vocab-sharded)
# --------------------------------------------------------------------------

def build_phase_b():
    nc = bacc.Bacc("TRN2", num_devices=NC, debug=False)
    hbT = nc.dram_tensor("hbT", [4, 128, BT], BF16, kind="ExternalInput").ap()
    wout = nc.dram_tensor("wout", [4, 128, VSL], BF16,
                          kind="ExternalInput").ap()
    boutT = nc.dram_tensor("boutT", [128, 30], F32,
                           kind="ExternalInput").ap()
    logits = nc.dram_tensor("logits", [VSL, BT], BF16,
                            kind="ExternalOutput").ap()

    vtiles = [(vt, vt * 128, min(128, VSL - vt * 128)) for vt in range(30)]
    nchunks = [(0, 512), (512, 512), (1024, 256)]

    with tile.TileContext(nc) as tc:
        with (
            tc.tile_pool(name="w", bufs=1) as wp,
            tc.tile_pool(name="l", bufs=4) as lp,
            tc.tile_pool(name="ps", bufs=2, space="PSUM") as ps,
        ):
            sb_hbT = wp.tile([128, 4, BT], BF16, tag="hbT")
            for k in range(4):
                for h in range(2):
                    nc.sync.dma_start(out=sb_hbT[:, k, ts(h, BT // 2)],
                                      in_=hbT[k, :, ts(h, BT // 2)])
            sb_wout = wp.tile([128, 4, VSL], BF16, tag="wout")
            vg = [(0, 960), (960, 960), (1920, 960), (2880, 870)]
            for g0, gw in vg:
                for k in range(4):
                    nc.sync.dma_start(out=sb_wout[:, k, g0:g0 + gw],
                                      in_=wout[k, :, g0:g0 + gw])
            sb_boutT = wp.tile([128, 30], F32, tag="boutT")
            nc.sync.dma_start(out=sb_boutT[:], in_=boutT)

            for vt, v0, vw in vtiles:
                pt = ps.tile([128, BT], F32, tag="acc")
                for n0, nw in nchunks:
                    for k in range(4):
                        nc.tensor.matmul(pt[:vw, n0:n0 + nw],
                                         sb_wout[:, k, v0:v0 + vw],
                                         sb_hbT[:, k, n0:n0 + nw],
                                         start=(k == 0), stop=(k == 3),
                                         skip_group_check=True)
                ot = lp.tile([128, BT], BF16, tag="out")
                if vt % 2 == 0:
                    nc.scalar.activation(out=ot[:vw], in_=pt[:vw],
                                         func=AF.Identity,
                                         bias=sb_boutT[:vw, vt:vt + 1])
                else:
                    nc.vector.tensor_scalar(
                        out=ot[:vw], in0=pt[:vw],
                        scalar1=sb_boutT[:vw, vt:vt + 1], scalar2=None,
                        op0=ALU.add)
                nc.sync.dma_start(out=logits[v0:v0 + vw], in_=ot[:vw])
    nc.compile()
    return nc


# --------------------------------------------------------------------------
# Host-side preparation + driver
# --------------------------------------------------------------------------

def prep_phase_a_inputs(features, encoder_out, emb, W_enc, b_enc, W_dec, b_dec,
                        v_w, W_g0, b_g0, W_g1, b_g1, W_ih, b_ih, W_ic, b_ic,
                        captions):
    embeds = np.asarray(emb)[np.asarray(captions)[:, :T].astype(np.int64)]
    cs = np.ones((G4,), np.float32)     # sigmoid halving on f, i, o columns
    cs[0:H] = 0.5
    cs[H:2 * H] = 0.5
    cs[3 * H:4 * H] = 0.5
    W_g0 = np.asarray(W_g0) * cs
    W_g1 = np.asarray(W_g1) * cs
    shared = {
        "wih2": _bf16(2.0 * np.asarray(W_ih).reshape(4, 128, H)),
        "wic2": _bf16(2.0 * np.asarray(W_ic).reshape(4, 128, H)),
        "bih2": _bf16(2.0 * np.asarray(b_ih).reshape(1, H)),
        "bic2": _bf16(2.0 * np.asarray(b_ic).reshape(1, H)),
        "wenc": _bf16(np.asarray(W_enc).reshape(4, 128, A)),
        "biasadT": _bf16((np.asarray(b_enc) + np.asarray(b_dec))
                         .reshape(4, 128, 1)),
        "wdech": _bf16(0.5 * np.asarray(W_dec).reshape(4, 128, A)),
        "vcol": _bf16(np.asarray(v_w).reshape(4, 128, 1)),
        "wg0x": _bf16(W_g0[:E].reshape(4, 128, G4)),
        "bg0": _bf16((np.asarray(b_g0) * cs).reshape(1, G4)),
        "wg0c": _bf16(W_g0[E:2 * E].reshape(4, 128, G4)),
        "wg0h": _bf16(0.5 * W_g0[2 * E:].reshape(4, 128, G4)),
        "wg1a": _bf16(0.5 * W_g1[:H].reshape(4, 128, G4)),
        "wg1b": _bf16(0.5 * W_g1[H:].reshape(4, 128, G4)),
        "bg1": _bf16((np.asarray(b_g1) * cs).reshape(1, G4)),
    }
    in_maps = []
    for c in range(NC):
        bs = slice(c * BL, (c + 1) * BL)
        enc = np.asarray(encoder_out)[bs]               # [8, 196, 512]
        encTn = enc.transpose(2, 1, 0).reshape(E, BP)   # (e, p, b)
        encpad = np.zeros((BL, PPAD, E), np.float32)
        encpad[:, :P] = enc
        feat = np.asarray(features)[bs]
        we = embeds[bs]                                 # [8, T, E]
        m = dict(shared)
        m["encT"] = _bf16(encTn.reshape(4, 128, BP))
        m["encflat"] = _bf16(encpad.reshape(NF, 128, E))
        m["featT"] = _bf16(feat.T.reshape(4, 128, BL))
        m["weT"] = _bf16(we.transpose(2, 1, 0).reshape(4, 128, T * BL))
        in_maps.append(m)
    return in_maps


_CACHE = {}


def kernel(**inputs):
    inputs = {k: np.asarray(v) for k, v in inputs.items()}
    if "a" not in _CACHE:
        _CACHE["a"] = build_phase_a()
    if "b" not in _CACHE:
        _CACHE["b"] = build_phase_b()

    in_a = prep_phase_a_inputs(
        inputs["features"], inputs["encoder_out"], inputs["emb"],
        inputs["W_enc"], inputs["b_enc"], inputs["W_dec"], inputs["b_dec"],
        inputs["v_w"], inputs["W_g0"], inputs["b_g0"], inputs["W_g1"],
        inputs["b_g1"], inputs["W_ih"], inputs["b_ih"], inputs["W_ic"],
        inputs["b_ic"], inputs["captions"])
    ra = run_bass_kernel_spmd(_CACHE["a"], in_a, core_ids=list(range(NC)))

    # reassemble hb: column index b*T + t
    hbT_full = np.zeros((4, 128, BT), dtype=ml_dtypes.bfloat16)
    for c in range(NC):
        part = ra.results[c]["hballT"].reshape(4, 128, T, BL)
        for bl in range(BL):
            b = c * BL + bl
            hbT_full[:, :, b * T:(b + 1) * T] = part[:, :, :, bl]

    W_out = np.asarray(inputs["W_out"])
    b_out = np.asarray(inputs["b_out"])
    bpad = np.zeros((30 * 128,), np.float32)
    in_b = []
    for c in range(NC):
        vs = slice(c * VSL, (c + 1) * VSL)
        bpad[:VSL] = b_out[vs]
        in_b.append({
            "hbT": hbT_full,
            "wout": _bf16(0.5 * W_out[:, vs].reshape(4, 128, VSL)),
            "boutT": np.ascontiguousarray(bpad.reshape(30, 128).T,
                                          dtype=np.float32),
        })
    rb = run_bass_kernel_spmd(_CACHE["b"], in_b, core_ids=list(range(NC)))
    out = np.empty((BT, V), np.float32)
    for c in range(NC):
        vs = slice(c * VSL, (c + 1) * VSL)
        out[:, vs] = rb.results[c]["logits"].astype(np.float32).T
    return out.reshape(B, T, V)


# revision 42
# speedup vs baseline: 1.0470x; 1.0470x over previous
"""Trainium2 Bass kernel for nn_CaptionDecoder (attention LSTM caption decoder).

Strategy (8 NeuronCores):
  Phase A: data-parallel over batch (8 batches/core) for the sequential
           attention + 2-layer-LSTM recurrence. Every per-step matmul is in
           transposed orientation (feature dim on PSUM partitions, batch as
           the streamed free dim), so gates, dec_proj, attention scores and
           context all come out of PSUM already transposed and the LSTM
           pointwise runs on 128-partition tiles; there are no per-step DMAs
           and no per-step PE transposes. The 8 batches are split into two
           independent 4-batch streams, software-pipelined with a half-step
           skew so one stream's attention tanh (the scalar-engine floor)
           overlaps the other stream's matmul/vector segments.
  Host:    gathers top-layer hidden states hb from the 8 cores (1.3 MB).
  Phase B: vocab-parallel logits projection in transposed layout
           (vocab tile on partitions, all 1280 (b,t) rows streamed), so
           b_out becomes a per-partition bias folded into the PSUM
           evacuation, which alternates between ACT and DVE to stay off
           the tensor-engine critical path.

Precision: bf16 matmuls with fp32 PSUM accumulation; fp32 cell state.
Sigmoid is computed as 0.5*(1+tanh(x/2)); the 0.5 factors are folded into
host-prescaled weights (column scale on f,i,o gates) and doubled states
H2=2h, C2=2c (row scale on recurrent weights), so one tanh over all 2048
gate pre-activations covers every gate nonlinearity.
"""

import numpy as np
import ml_dtypes

import concourse.bass as bass
import concourse.bacc as bacc
import concourse.mybir as mybir
import concourse.tile as tile
from concourse.bass import ts
from concourse.bass_utils import run_bass_kernel_spmd
from concourse.masks import make_identity

F32 = mybir.dt.float32
BF16 = mybir.dt.bfloat16
AF = mybir.ActivationFunctionType
ALU = mybir.AluOpType

B, TC, P, E, H, A, V = 64, 21, 196, 512, 512, 512, 30000
T = TC - 1            # 20 decode steps
NC = 8                # cores
BL = B // NC          # 8 batches per core
PPAD = 256            # padded attention positions per batch
NF = BL * PPAD // 128  # 16 (batch, p-half) chunks
BP = BL * P           # 1568 (p, b) columns per core
G4 = 4 * H            # 2048 stacked gates f,i,C,o
VSL = V // NC         # 3750 vocab rows per core (phase B)
BT = B * T            # 1280 (b, t) columns
P1 = P - 128          # 68 positions in the second p-half


def _bf16(x):
    return np.ascontiguousarray(np.asarray(x, dtype=np.float32)).astype(
        ml_dtypes.bfloat16)


def _sub(ap, dims, extra_offset=0):
    """Custom free-dim access pattern on an AP, keeping its partition dim."""
    return bass.AP(ap.tensor, ap.offset + extra_offset,
                   [list(ap.ap[0])] + [list(d) for d in dims])


def _pbcast(ap, dims, extra_offset=0):
    """Partition-broadcast (stride 0) custom AP."""
    return bass.AP(ap.tensor, ap.offset + extra_offset,
                   [[0, 128]] + [list(d) for d in dims])


# --------------------------------------------------------------------------
# Phase A module: the recurrence
# --------------------------------------------------------------------------

DBG = False


def build_phase_a(n_steps=T):
    nc = bacc.Bacc("TRN2", num_devices=NC, debug=False)

    def din(name, shape, dt=BF16):
        return nc.dram_tensor(name, shape, dt, kind="ExternalInput").ap()

    encT = din("encT", [4, 128, BP])          # encoder_out^T  [a-chk][a][(p,b)]
    encflat = din("encflat", [NF, 128, E])    # [(b,half)][p][e], 0-padded
    featT = din("featT", [4, 128, BL])
    wih2 = din("wih2", [4, 128, H])           # 2*W_ih
    wic2 = din("wic2", [4, 128, H])           # 2*W_ic
    bih2 = din("bih2", [1, H])
    bic2 = din("bic2", [1, H])
    wenc = din("wenc", [4, 128, A])
    biasadT = din("biasadT", [4, 128, 1])     # b_enc + b_dec, transposed
    wdech = din("wdech", [4, 128, A])         # 0.5*W_dec
    vcol = din("vcol", [4, 128, 1])
    weT = din("weT", [4, 128, T * BL])        # embeds^T, col = t*8+b
    wg0x = din("wg0x", [4, 128, G4])          # col-scaled
    bg0 = din("bg0", [1, G4])
    wg0c = din("wg0c", [4, 128, G4])
    wg0h = din("wg0h", [4, 128, G4])
    wg1a = din("wg1a", [4, 128, G4])
    wg1b = din("wg1b", [4, 128, G4])
    bg1 = din("bg1", [1, G4])

    hballT = nc.dram_tensor("hballT", [4, 128, T * BL], BF16,
                            kind="ExternalOutput").ap()
    if DBG:
        dbg = {
            "dbg_h0": nc.dram_tensor("dbg_h0", [128, 32], BF16,
                                     kind="ExternalOutput").ap(),
            "dbg_c0": nc.dram_tensor("dbg_c0", [128, 32], F32,
                                     kind="ExternalOutput").ap(),
            "dbg_encproj": nc.dram_tensor("dbg_encproj", [128, 4 * BP], BF16,
                                          kind="ExternalOutput").ap(),
            "dbg_decT": nc.dram_tensor("dbg_decT", [128, 32], BF16,
                                       kind="ExternalOutput").ap(),
            "dbg_ssb": nc.dram_tensor("dbg_ssb", [128, 4 * BP], BF16,
                                      kind="ExternalOutput").ap(),
            "dbg_aB": nc.dram_tensor("dbg_aB", [128, 16], BF16,
                                     kind="ExternalOutput").ap(),
            "dbg_ctxT": nc.dram_tensor("dbg_ctxT", [128, 32], BF16,
                                       kind="ExternalOutput").ap(),
            "dbg_tg0": nc.dram_tensor("dbg_tg0", [128, 128], BF16,
                                      kind="ExternalOutput").ap(),
            "dbg_wep": nc.dram_tensor("dbg_wep", [128, 16 * T * BL], BF16,
                                      kind="ExternalOutput").ap(),
        }

    with tile.TileContext(nc) as tc:
        with (
            tc.tile_pool(name="persist", bufs=1) as pp,
            tc.tile_pool(name="lp", bufs=2) as lp,
            tc.tile_pool(name="pa", bufs=1, space="PSUM") as pa,
            tc.tile_pool(name="pg", bufs=1, space="PSUM") as pg,
        ):
            def dma3(dst, src, n=4):  # dram [n,128,X] -> sbuf [128,n,X]
                for k in range(n):
                    nc.sync.dma_start(out=dst[:, k], in_=src[k])

            # ---- loads ordered by first use (DMA queues drain in order) --
            sp = tc.alloc_tile_pool(name="stream", bufs=1)
            sq = tc.alloc_tile_pool(name="sq", bufs=2, space="PSUM")
            sb_featT = sp.tile([128, 4, BL], BF16, tag="featT")
            dma3(sb_featT, featT)
            sb_wih2 = sp.tile([128, 4, H], BF16, tag="wih2")
            dma3(sb_wih2, wih2)
            sb_wic2 = sp.tile([128, 4, H], BF16, tag="wic2")
            dma3(sb_wic2, wic2)
            sb_bih2 = sp.tile([1, H], BF16, tag="bih2")
            nc.sync.dma_start(out=sb_bih2[:], in_=bih2)
            sb_bic2 = sp.tile([1, H], BF16, tag="bic2")
            nc.sync.dma_start(out=sb_bic2[:], in_=bic2)
            sb_encT = sp.tile([128, 4, BP], BF16, tag="encT")
            dma3(sb_encT, encT)
            sb_wenc = sp.tile([128, 4, A], BF16, tag="wenc")
            dma3(sb_wenc, wenc)
            sb_wdech = pp.tile([128, 4, A], BF16, tag="wdech")
            dma3(sb_wdech, wdech)
            sb_vcol = pp.tile([128, 4, 1], BF16, tag="vcol")
            dma3(sb_vcol, vcol)
            sb_biasadT = pp.tile([128, 4, 1], BF16, tag="biasadT")
            dma3(sb_biasadT, biasadT)
            sb_weT = sp.tile([128, 4, T * BL], BF16, tag="weT")
            dma3(sb_weT, weT)
            sb_wg0x = sp.tile([128, 4, G4], BF16, tag="wg0x")
            dma3(sb_wg0x, wg0x)
            sb_bg0 = sp.tile([1, G4], BF16, tag="bg0")
            nc.sync.dma_start(out=sb_bg0[:], in_=bg0)
            sb_encflat = pp.tile([128, NF, E], BF16, tag="encflat")
            dma3(sb_encflat, encflat, n=NF)
            sb_wg0h = pp.tile([128, 4, G4], BF16, tag="wg0h")
            dma3(sb_wg0h, wg0h)
            sb_wg0c = pp.tile([128, 4, G4], BF16, tag="wg0c")
            dma3(sb_wg0c, wg0c)
            sb_bg1 = pp.tile([1, G4], BF16, tag="bg1")
            nc.sync.dma_start(out=sb_bg1[:], in_=bg1)
            sb_wg1b = pp.tile([128, 4, G4], BF16, tag="wg1b")
            dma3(sb_wg1b, wg1b)
            sb_wg1a = pp.tile([128, 4, G4], BF16, tag="wg1a")
            dma3(sb_wg1a, wg1a)

            i128 = pp.tile([128, 128], BF16, tag="i128")
            make_identity(nc, i128[:])
            ones_1x8 = pp.tile([1, 8], BF16, tag="o18")
            nc.vector.memset(ones_1x8[:], 1.0)
            ones_1xT8 = pp.tile([1, T * BL], BF16, tag="o1T8")
            nc.vector.memset(ones_1xT8[:], 1.0)
            ones_col = pp.tile([128, 1], BF16, tag="ocol")
            nc.vector.memset(ones_col[:], 1.0)
            ones_1x128f = pp.tile([1, 128], F32, tag="o1128f")
            nc.vector.memset(ones_1x128f[:], 1.0)

            # persistent state, two independent 4-batch streams
            # (all transposed: [128, 4, 4] = [dim%128, dim//128, batch-in-stream])
            BS = BL // 2
            C2aT = [pp.tile([128, 4, BS], F32, tag=f"C2aT{s}", name=f"C2aT{s}")
                    for s in (0, 1)]
            C2bT = [pp.tile([128, 4, BS], F32, tag=f"C2bT{s}", name=f"C2bT{s}")
                    for s in (0, 1)]
            H2aT = [pp.tile([128, 4, BS], BF16, tag=f"H2aT{s}", name=f"H2aT{s}")
                    for s in (0, 1)]
            H2b0 = [pp.tile([128, 4, BS], BF16, tag=f"H2b0{s}", name=f"H2b0{s}")
                    for s in (0, 1)]
            hball = pp.tile([128, 4, T * BL], BF16, tag="hball")

            aB = [pp.tile([128, 2 * BS], BF16, tag=f"aB{s}", name=f"aB{s}")
                  for s in (0, 1)]
            nc.vector.memset(aB[0][:], 0.0)
            nc.vector.memset(aB[1][:], 0.0)

            encproj = pp.tile([128, 4, BP], BF16, tag="encproj")
            wepartT = pp.tile([128, 16, T * BL], BF16, tag="wepartT")

            # ---- one-time section ----------------------------------------
            # h0/c0 transposed: out[h%128, hc, b]
            hc0 = sq.tile([128, 512], F32, tag="ot", name="hc0")
            for (w_sb, b_sb, co) in ((sb_wih2, sb_bih2, 0),
                                     (sb_wic2, sb_bic2, 32)):
                for hc in range(4):
                    reg = hc0[:, co + hc * 8:co + (hc + 1) * 8]
                    for k in range(4):
                        nc.tensor.matmul(reg, w_sb[:, k, ts(hc, 128)],
                                         sb_featT[:, k],
                                         start=(k == 0), stop=False,
                                         skip_group_check=True)
                    nc.tensor.matmul(reg, b_sb[:, ts(hc, 128)], ones_1x8[:],
                                     start=False, stop=True,
                                     skip_group_check=True)
            for s in (0, 1):
                h_sl = _sub(hc0[:], [[8, 4], [1, BS]], extra_offset=s * BS)
                c_sl = _sub(hc0[:], [[8, 4], [1, BS]],
                            extra_offset=32 + s * BS)
                nc.scalar.activation(out=H2aT[s][:], in_=h_sl, func=AF.Copy)
                nc.scalar.activation(out=H2b0[s][:], in_=h_sl, func=AF.Copy)
                nc.vector.tensor_copy(C2aT[s][:], c_sl)
                nc.vector.tensor_copy(C2bT[s][:], c_sl)

            # ---- one-time: enc_projT (+ bias) ----------------------------
            QS = BP // 4  # 392
            for k in range(4):          # a-chunk
                for q in range(4):      # (p,b) quarter
                    eps = sq.tile([128, 512], F32, tag="ot", name="eps")
                    for e in range(4):  # e-chunk
                        nc.tensor.matmul(
                            eps[:, 0:QS], sb_wenc[:, e, ts(k, 128)],
                            sb_encT[:, e, ts(q, QS)],
                            start=(e == 0), stop=(e == 3))
                    nc.scalar.activation(out=encproj[:, k, ts(q, QS)],
                                         in_=eps[:, 0:QS], func=AF.Identity,
                                         bias=sb_biasadT[:, k])


            # ================= the recurrent steps ========================
            # Two 4-batch streams per core; emission ping-pongs between them
            # so one stream's big ACT tanh overlaps the other's DVE/PE work.
            def pointwise(nc, lp, gps, s, C2T, h2out, tag):
                """gps = strided [128, 16, BS] view of the gates psum."""
                tg = lp.tile([128, 16, BS], BF16, tag="tg" + tag)
                nc.scalar.activation(
                    out=tg[:].rearrange("p g b -> p (g b)"), in_=gps,
                    func=AF.Tanh)
                tf = _sub(tg[:], [[BS, 4], [1, BS]], extra_offset=0)
                ti = _sub(tg[:], [[BS, 4], [1, BS]], extra_offset=4 * BS)
                tC = _sub(tg[:], [[BS, 4], [1, BS]], extra_offset=8 * BS)
                to = _sub(tg[:], [[BS, 4], [1, BS]], extra_offset=12 * BS)
                s1 = lp.tile([128, 4, BS], F32, tag="s1" + tag)
                nc.vector.scalar_tensor_tensor(
                    out=s1[:], in0=tf, scalar=1.0, in1=C2T[:],
                    op0=ALU.add, op1=ALU.mult)
                s2 = lp.tile([128, 4, BS], F32, tag="s2" + tag)
                nc.vector.scalar_tensor_tensor(
                    out=s2[:], in0=ti, scalar=1.0, in1=tC,
                    op0=ALU.add, op1=ALU.mult)
                nc.vector.scalar_tensor_tensor(
                    out=C2T[:], in0=s1[:], scalar=0.5, in1=s2[:],
                    op0=ALU.mult, op1=ALU.add)
                tch = lp.tile([128, 4, BS], BF16, tag="tch" + tag)
                nc.scalar.activation(out=tch[:], in_=C2T[:],
                                     func=AF.Tanh, scale=0.5)
                nc.vector.scalar_tensor_tensor(
                    out=h2out, in0=to, scalar=1.0, in1=tch[:],
                    op0=ALU.add, op1=ALU.mult)

            # ---- one-time: wepartT = (we @ Wg0x + bg0)^T -----------------
            for gc in range(16):
                wps = sq.tile([128, 512], F32, tag="ot", name="wps")
                nc.tensor.matmul(wps[:, 0:T * BL], sb_bg0[:, ts(gc, 128)],
                                 ones_1xT8[:], start=True, stop=False)
                for k in range(4):
                    nc.tensor.matmul(wps[:, 0:T * BL],
                                     sb_wg0x[:, k, ts(gc, 128)],
                                     sb_weT[:, k],
                                     start=False, stop=(k == 3))
                nc.vector.tensor_copy(wepartT[:, gc], wps[:, 0:T * BL])

            sp.release()
            sq.release()

            # ---- per-op emitters; state dicts keyed by stream ------------
            ST = [{"ps": None, "decT": None, "ssb": None, "rinvr": None,
                   "ctxT": None, "g0": None, "g1": None} for _ in (0, 1)]

            def H2bTk(s, k, t):
                if t == 0:
                    return H2b0[s][:, k]
                return _sub(hball[:], [[1, BS]],
                            extra_offset=k * T * BL + (t - 1) * BL + s * BS)

            def e_dec(s, t):
                st = ST[s]
                st["ps"] = pa.tile([128, 128], F32, tag=f"ps{s}",
                                   name=f"ps{s}")
                for ac in range(4):
                    reg = st["ps"][:, ac * BS:(ac + 1) * BS]
                    for k in range(4):
                        nc.tensor.matmul(reg, sb_wdech[:, k, ts(ac, 128)],
                                         H2bTk(s, k, t),
                                         start=(k == 0), stop=(k == 3),
                                         skip_group_check=True)

            def e_decT(s, k):
                st = ST[s]
                if k != 0:
                    return
                st["decT"] = lp.tile([128, 4, BS], BF16, tag=f"decT{s}",
                                     name=f"decT{s}")
                st["ssb"] = lp.tile([128, 4, P * BS], BF16,
                                    tag=f"ssb{s}", name=f"ssb{s}")
                nc.scalar.activation(
                    out=st["decT"][:].rearrange("p a b -> p (a b)"),
                    in_=st["ps"][:, 0:16], func=AF.Copy)

            def e_add(s, k):
                st = ST[s]
                nc.vector.tensor_tensor(
                    st["ssb"][:, k].rearrange("p (q b) -> p q b", b=BS),
                    _sub(encproj[:, k], [[BL, P], [1, BS]],
                         extra_offset=s * BS),
                    _sub(st["decT"][:], [[0, P], [1, BS]],
                         extra_offset=k * BS),
                    ALU.add)

            def e_tanh(s, k):
                nc.scalar.activation(out=ST[s]["ssb"][:, k],
                                     in_=ST[s]["ssb"][:, k], func=AF.Tanh)

            def e_scores(s):
                st = ST[s]
                for half, pw, poff in ((0, 128, 0), (1, P1, 128 * BS)):
                    for j in range(BS):
                        col = 32 + half * BS + j
                        for k in range(4):
                            nc.tensor.matmul(
                                st["ps"][:pw, col:col + 1],
                                _sub(st["ssb"][:, k], [[BS, pw]],
                                     extra_offset=poff + j),
                                sb_vcol[:, k],
                                start=(k == 0), stop=(k == 3),
                                skip_group_check=True)

            def e_exp(s):
                pst = ST[s]["ps"]
                nc.scalar.activation(out=aB[s][:, 0:BS],
                                     in_=pst[:, 32:32 + BS], func=AF.Exp)
                nc.scalar.activation(out=aB[s][0:P1, BS:2 * BS],
                                     in_=pst[0:P1, 32 + BS:32 + 2 * BS],
                                     func=AF.Exp)

            def e_sums(s):
                pst = ST[s]["ps"]
                nc.tensor.matmul(pst[0:1, 48:48 + BS], ones_col[:],
                                 aB[s][:, 0:BS], start=True, stop=False,
                                 skip_group_check=True)
                nc.tensor.matmul(pst[0:1, 48:48 + BS], ones_col[:],
                                 aB[s][:, BS:2 * BS], start=False,
                                 stop=True, skip_group_check=True)

            def e_recip(s):
                st = ST[s]
                st["rinvr"] = lp.tile([1, BS], F32, tag=f"rinvr{s}",
                                      name=f"rinvr{s}")
                nc.vector.reciprocal(st["rinvr"][:],
                                     st["ps"][0:1, 48:48 + BS])

            def e_ctx(s):
                pst = ST[s]["ps"]
                for j in range(BS):
                    b = s * BS + j
                    for ec in range(4):
                        col = 64 + ec * BS + j
                        for half in (0, 1):
                            nc.tensor.matmul(
                                pst[:, col:col + 1],
                                sb_encflat[:, 2 * b + half, ts(ec, 128)],
                                aB[s][:, half * BS + j:half * BS + j + 1],
                                start=(half == 0), stop=(half == 1),
                                skip_group_check=True)

            def e_ctxT(s):
                st = ST[s]
                rb = lp.tile([128, BS], F32, tag=f"rb{s}", name=f"rb{s}")
                nc.gpsimd.partition_broadcast(rb[:], st["rinvr"][:])
                st["ctxT"] = lp.tile([128, 4, BS], BF16, tag=f"ctxT{s}",
                                     name=f"ctxT{s}")
                nc.vector.tensor_tensor(
                    st["ctxT"][:],
                    _sub(st["ps"][:], [[BS, 4], [1, BS]], extra_offset=64),
                    _sub(rb[:], [[0, 4], [1, BS]]),
                    ALU.mult)

            def e_g0(s, t):
                st = ST[s]
                st["g0"] = pg.tile([128, 64], F32, tag=f"g0{s}",
                                   name=f"g0{s}")
                for gc in range(16):
                    reg = st["g0"][:, gc * BS:(gc + 1) * BS]
                    nc.tensor.matmul(
                        reg, i128[:],
                        wepartT[:, gc,
                                t * BL + s * BS:t * BL + (s + 1) * BS],
                        start=True, stop=False)
                    for k in range(4):
                        nc.tensor.matmul(reg, sb_wg0h[:, k, ts(gc, 128)],
                                         H2aT[s][:, k], start=False,
                                         stop=False)
                    for k in range(4):
                        nc.tensor.matmul(reg, sb_wg0c[:, k, ts(gc, 128)],
                                         st["ctxT"][:, k], start=False,
                                         stop=(k == 3))

            def e_pw0(s):
                pointwise(nc, lp, _sub(ST[s]["g0"][:], [[BS, 16], [1, BS]]),
                          s, C2aT[s], H2aT[s][:], f"0{s}")

            def e_g1(s, t):
                st = ST[s]
                st["g1"] = pg.tile([128, 64], F32, tag=f"g1{s}",
                                   name=f"g1{s}")
                for gc in range(16):
                    reg = st["g1"][:, gc * BS:(gc + 1) * BS]
                    nc.tensor.matmul(reg, sb_bg1[:, ts(gc, 128)],
                                     ones_1x8[:, 0:BS],
                                     start=True, stop=False)
                    for k in range(4):
                        nc.tensor.matmul(reg, sb_wg1b[:, k, ts(gc, 128)],
                                         H2bTk(s, k, t), start=False,
                                         stop=False)
                    for k in range(4):
                        nc.tensor.matmul(reg, sb_wg1a[:, k, ts(gc, 128)],
                                         H2aT[s][:, k], start=False,
                                         stop=(k == 3))

            def e_pw1(s, t):
                h2b_out = _sub(hball[:], [[T * BL, 4], [1, BS]],
                               extra_offset=t * BL + s * BS)
                pointwise(nc, lp, _sub(ST[s]["g1"][:], [[BS, 16], [1, BS]]),
                          s, C2bT[s], h2b_out, f"1{s}")

            def front(s, t):
                """dec + adds + tanh + scores, self-contained (prologue)."""
                e_dec(s, t)
                for k in range(4):
                    e_decT(s, k)
                    e_add(s, k)
                    e_tanh(s, k)
                e_scores(s)

            def slot(sb_, sf, t_back, t_front, do_front):
                """Emit stream sb_'s back-half of step t_back, micro-
                interleaved with stream sf's front-half of step t_front."""
                if do_front:
                    e_dec(sf, t_front)
                e_exp(sb_)
                if do_front:
                    e_decT(sf, 0)
                    e_add(sf, 0)
                e_sums(sb_)
                e_recip(sb_)
                if do_front:
                    e_tanh(sf, 0)
                    e_decT(sf, 1)
                    e_add(sf, 1)
                e_ctx(sb_)
                e_ctxT(sb_)
                if do_front:
                    e_tanh(sf, 1)
                    e_decT(sf, 2)
                e_g0(sb_, t_back)
                if do_front:
                    e_add(sf, 2)
                e_pw0(sb_)
                if do_front:
                    e_tanh(sf, 2)
                    e_decT(sf, 3)
                    e_add(sf, 3)
                e_g1(sb_, t_back)
                if do_front:
                    e_tanh(sf, 3)
                e_pw1(sb_, t_back)
                if do_front:
                    e_scores(sf)

            # ---- software-pipelined schedule: stream 1 lags half a step --
            front(0, 0)
            for t in range(n_steps):
                slot(0, 1, t, t, True)                       # s0 back | s1 front
                slot(1, 0, t, t + 1, t + 1 < n_steps)        # s1 back | s0 front

            for k in range(4):
                nc.sync.dma_start(out=hballT[k], in_=hball[:, k])

    nc.compile()
    return nc


# --------------------------------------------------------------------------
# Phase B module: logitsT = (0.5*W_out)^T @ H2b_all + b_out  (vocab-sharded)
# --------------------------------------------------------------------------

def build_phase_b():
    nc = bacc.Bacc("TRN2", num_devices=NC, debug=False)
    hbT = nc.dram_tensor("hbT", [4, 128, BT], BF16, kind="ExternalInput").ap()
    wout = nc.dram_tensor("wout", [4, 128, VSL], BF16,
                          kind="ExternalInput").ap()
    boutT = nc.dram_tensor("boutT", [128, 30], F32,
                           kind="ExternalInput").ap()
    logits = nc.dram_tensor("logits", [VSL, BT], BF16,
                            kind="ExternalOutput").ap()

    vtiles = [(vt, vt * 128, min(128, VSL - vt * 128)) for vt in range(30)]
    nchunks = [(0, 512), (512, 512), (1024, 256)]

    with tile.TileContext(nc) as tc:
        with (
            tc.tile_pool(name="w", bufs=1) as wp,
            tc.tile_pool(name="l", bufs=4) as lp,
            tc.tile_pool(name="ps", bufs=2, space="PSUM") as ps,
        ):
            sb_hbT = wp.tile([128, 4, BT], BF16, tag="hbT")
            for k in range(4):
                for h in range(2):
                    nc.sync.dma_start(out=sb_hbT[:, k, ts(h, BT // 2)],
                                      in_=hbT[k, :, ts(h, BT // 2)])
            sb_wout = wp.tile([128, 4, VSL], BF16, tag="wout")
            vg = [(0, 960), (960, 960), (1920, 960), (2880, 870)]
            for g0, gw in vg:
                for k in range(4):
                    nc.sync.dma_start(out=sb_wout[:, k, g0:g0 + gw],
                                      in_=wout[k, :, g0:g0 + gw])
            sb_boutT = wp.tile([128, 30], F32, tag="boutT")
            nc.sync.dma_start(out=sb_boutT[:], in_=boutT)

            for vt, v0, vw in vtiles:
                pt = ps.tile([128, BT], F32, tag="acc")
                for n0, nw in nchunks:
                    for k in range(4):
                        nc.tensor.matmul(pt[:vw, n0:n0 + nw],
                                         sb_wout[:, k, v0:v0 + vw],
                                         sb_hbT[:, k, n0:n0 + nw],
                                         start=(k == 0), stop=(k == 3),
                                         skip_group_check=True)
                ot = lp.tile([128, BT], BF16, tag="out")
                if vt % 2 == 0:
                    nc.scalar.activation(out=ot[:vw], in_=pt[:vw],
                                         func=AF.Identity,
                                         bias=sb_boutT[:vw, vt:vt + 1])
                else:
                    nc.vector.tensor_scalar(
                        out=ot[:vw], in0=pt[:vw],
                        scalar1=sb_boutT[:vw, vt:vt + 1], scalar2=None,
                        op0=ALU.add)
                nc.sync.dma_start(out=logits[v0:v0 + vw], in_=ot[:vw])
    nc.compile()
    return nc


# --------------------------------------------------------------------------
# Host-side preparation + driver
# --------------------------------------------------------------------------

def prep_phase_a_inputs(features, encoder_out, emb, W_enc, b_enc, W_dec, b_dec,
                        v_w, W_g0, b_g0, W_g1, b_g1, W_ih, b_ih, W_ic, b_ic,
                        captions):
    embeds = np.asarray(emb)[np.asarray(captions)[:, :T].astype(np.int64)]
    cs = np.ones((G4,), np.float32)     # sigmoid halving on f, i, o columns
    cs[0:H] = 0.5
    cs[H:2 * H] = 0.5
    cs[3 * H:4 * H] = 0.5
    W_g0 = np.asarray(W_g0) * cs
    W_g1 = np.asarray(W_g1) * cs
    shared = {
        "wih2": _bf16(2.0 * np.asarray(W_ih).reshape(4, 128, H)),
        "wic2": _bf16(2.0 * np.asarray(W_ic).reshape(4, 128, H)),
        "bih2": _bf16(2.0 * np.asarray(b_ih).reshape(1, H)),
        "bic2": _bf16(2.0 * np.asarray(b_ic).reshape(1, H)),
        "wenc": _bf16(np.asarray(W_enc).reshape(4, 128, A)),
        "biasadT": _bf16((np.asarray(b_enc) + np.asarray(b_dec))
                         .reshape(4, 128, 1)),
        "wdech": _bf16(0.5 * np.asarray(W_dec).reshape(4, 128, A)),
        "vcol": _bf16(np.asarray(v_w).reshape(4, 128, 1)),
        "wg0x": _bf16(W_g0[:E].reshape(4, 128, G4)),
        "bg0": _bf16((np.asarray(b_g0) * cs).reshape(1, G4)),
        "wg0c": _bf16(W_g0[E:2 * E].reshape(4, 128, G4)),
        "wg0h": _bf16(0.5 * W_g0[2 * E:].reshape(4, 128, G4)),
        "wg1a": _bf16(0.5 * W_g1[:H].reshape(4, 128, G4)),
        "wg1b": _bf16(0.5 * W_g1[H:].reshape(4, 128, G4)),
        "bg1": _bf16((np.asarray(b_g1) * cs).reshape(1, G4)),
    }
    in_maps = []
    for c in range(NC):
        bs = slice(c * BL, (c + 1) * BL)
        enc = np.asarray(encoder_out)[bs]               # [8, 196, 512]
        encTn = enc.transpose(2, 1, 0).reshape(E, BP)   # (e, p, b)
        encpad = np.zeros((BL, PPAD, E), np.float32)
        encpad[:, :P] = enc
        feat = np.asarray(features)[bs]
        we = embeds[bs]                                 # [8, T, E]
        m = dict(shared)
        m["encT"] = _bf16(encTn.reshape(4, 128, BP))
        m["encflat"] = _bf16(encpad.reshape(NF, 128, E))
        m["featT"] = _bf16(feat.T.reshape(4, 128, BL))
        m["weT"] = _bf16(we.transpose(2, 1, 0).reshape(4, 128, T * BL))
        in_maps.append(m)
    return in_maps


_CACHE = {}


def kernel(**inputs):
    inputs = {k: np.asarray(v) for k, v in inputs.items()}
    if "a" not in _CACHE:
        _CACHE["a"] = build_phase_a()
    if "b" not in _CACHE:
        _CACHE["b"] = build_phase_b()

    in_a = prep_phase_a_inputs(
        inputs["features"], inputs["encoder_out"], inputs["emb"],
        inputs["W_enc"], inputs["b_enc"], inputs["W_dec"], inputs["b_dec"],
        inputs["v_w"], inputs["W_g0"], inputs["b_g0"], inputs["W_g1"],
        inputs["b_g1"], inputs["W_ih"], inputs["b_ih"], inputs["W_ic"],
        inputs["b_ic"], inputs["captions"])
    ra = run_bass_kernel_spmd(_CACHE["a"], in_a, core_ids=list(range(NC)))

    # reassemble hb: column index b*T + t
    hbT_full = np.zeros((4, 128, BT), dtype=ml_dtypes.bfloat16)
    for c in range(NC):
        part = ra.results[c]["hballT"].reshape(4, 128, T, BL)
        for bl in range(BL):
            b = c * BL + bl
            hbT_full[:, :, b * T:(b + 1) * T] = part[:, :, :, bl]

    W_out = np.asarray(inputs["W_out"])
    b_out = np.asarray(inputs["b_out"])
    bpad = np.zeros((30 * 128,), np.float32)
    in_b = []
    for c in range(NC):
        vs = slice(c * VSL, (c + 1) * VSL)
        bpad[:VSL] = b_out[vs]
        in_b.append({
            "hbT": hbT_full,
            "wout": _bf16(0.5 * W_out[:, vs].reshape(4, 128, VSL)),
            "boutT": np.ascontiguousarray(bpad.reshape(30, 128).T,
                                          dtype=np.float32),
        })
    rb = run_bass_kernel_spmd(_CACHE["b"], in_b, core_ids=list(range(NC)))
    out = np.empty((BT, V), np.float32)
    for c in range(NC):
        vs = slice(c * VSL, (c + 1) * VSL)
        out[:, vs] = rb.results[c]["logits"].astype(np.float32).T
    return out.reshape(B, T, V)


# revision 45
# speedup vs baseline: 1.0581x; 1.0106x over previous
"""Trainium2 Bass kernel for nn_CaptionDecoder (attention LSTM caption decoder).

Strategy (8 NeuronCores):
  Phase A: data-parallel over batch (8 batches/core) for the sequential
           attention + 2-layer-LSTM recurrence. Every per-step matmul is in
           transposed orientation (feature dim on PSUM partitions, batch as
           the streamed free dim), so gates, dec_proj, attention scores and
           context all come out of PSUM already transposed and the LSTM
           pointwise runs on 128-partition tiles; there are no per-step DMAs
           and no per-step PE transposes. The 8 batches are split into two
           independent 4-batch streams, software-pipelined with a half-step
           skew so one stream's attention tanh (the scalar-engine floor)
           overlaps the other stream's matmul/vector segments.
  Host:    gathers top-layer hidden states hb from the 8 cores (1.3 MB).
  Phase B: vocab-parallel logits projection in transposed layout
           (vocab tile on partitions, all 1280 (b,t) rows streamed), so
           b_out becomes a per-partition bias folded into the PSUM
           evacuation, which alternates between ACT and DVE to stay off
           the tensor-engine critical path.

Precision: bf16 matmuls with fp32 PSUM accumulation; fp32 cell state.
Sigmoid is computed as 0.5*(1+tanh(x/2)); the 0.5 factors are folded into
host-prescaled weights (column scale on f,i,o gates) and doubled states
H2=2h, C2=2c (row scale on recurrent weights), so one tanh over all 2048
gate pre-activations covers every gate nonlinearity.
"""

import numpy as np
import ml_dtypes

import concourse.bass as bass
import concourse.bacc as bacc
import concourse.mybir as mybir
import concourse.tile as tile
from concourse.bass import ts
from concourse.bass_utils import run_bass_kernel_spmd
from concourse.masks import make_identity

F32 = mybir.dt.float32
BF16 = mybir.dt.bfloat16
AF = mybir.ActivationFunctionType
ALU = mybir.AluOpType

B, TC, P, E, H, A, V = 64, 21, 196, 512, 512, 512, 30000
T = TC - 1            # 20 decode steps
NC = 8                # cores
BL = B // NC          # 8 batches per core
PPAD = 256            # padded attention positions per batch
NF = BL * PPAD // 128  # 16 (batch, p-half) chunks
BP = BL * P           # 1568 (p, b) columns per core
G4 = 4 * H            # 2048 stacked gates f,i,C,o
VSL = V // NC         # 3750 vocab rows per core (phase B)
BT = B * T            # 1280 (b, t) columns
P1 = P - 128          # 68 positions in the second p-half


def _bf16(x):
    return np.ascontiguousarray(np.asarray(x, dtype=np.float32)).astype(
        ml_dtypes.bfloat16)


def _sub(ap, dims, extra_offset=0):
    """Custom free-dim access pattern on an AP, keeping its partition dim."""
    return bass.AP(ap.tensor, ap.offset + extra_offset,
                   [list(ap.ap[0])] + [list(d) for d in dims])


def _pbcast(ap, dims, extra_offset=0):
    """Partition-broadcast (stride 0) custom AP."""
    return bass.AP(ap.tensor, ap.offset + extra_offset,
                   [[0, 128]] + [list(d) for d in dims])


# --------------------------------------------------------------------------
# Phase A module: the recurrence
# --------------------------------------------------------------------------

DBG = False


def build_phase_a(n_steps=T):
    nc = bacc.Bacc("TRN2", num_devices=NC, debug=False)

    def din(name, shape, dt=BF16):
        return nc.dram_tensor(name, shape, dt, kind="ExternalInput").ap()

    encT = din("encT", [4, 128, BP])          # encoder_out^T  [a-chk][a][(p,b)]
    encflat = din("encflat", [NF, 128, E])    # [(b,half)][p][e], 0-padded
    featT = din("featT", [4, 128, BL])
    wih2 = din("wih2", [4, 128, H])           # 2*W_ih
    wic2 = din("wic2", [4, 128, H])           # 2*W_ic
    bih2 = din("bih2", [1, H])
    bic2 = din("bic2", [1, H])
    wenc = din("wenc", [4, 128, A])
    biasadT = din("biasadT", [4, 128, 1])     # b_enc + b_dec, transposed
    wdech = din("wdech", [4, 128, A])         # 0.5*W_dec
    vcol = din("vcol", [4, 128, 1])
    weT = din("weT", [4, 128, T * BL])        # embeds^T, col = t*8+b
    wg0x = din("wg0x", [4, 128, G4])          # col-scaled
    bg0 = din("bg0", [1, G4])
    wg0c = din("wg0c", [4, 128, G4])
    wg0h = din("wg0h", [4, 128, G4])
    wg1a = din("wg1a", [4, 128, G4])
    wg1b = din("wg1b", [4, 128, G4])
    bg1 = din("bg1", [1, G4])

    hballT = nc.dram_tensor("hballT", [4, 128, T * BL], BF16,
                            kind="ExternalOutput").ap()
    if DBG:
        dbg = {
            "dbg_h0": nc.dram_tensor("dbg_h0", [128, 32], BF16,
                                     kind="ExternalOutput").ap(),
            "dbg_c0": nc.dram_tensor("dbg_c0", [128, 32], F32,
                                     kind="ExternalOutput").ap(),
            "dbg_encproj": nc.dram_tensor("dbg_encproj", [128, 4 * BP], BF16,
                                          kind="ExternalOutput").ap(),
            "dbg_decT": nc.dram_tensor("dbg_decT", [128, 32], BF16,
                                       kind="ExternalOutput").ap(),
            "dbg_ssb": nc.dram_tensor("dbg_ssb", [128, 4 * BP], BF16,
                                      kind="ExternalOutput").ap(),
            "dbg_aB": nc.dram_tensor("dbg_aB", [128, 16], BF16,
                                     kind="ExternalOutput").ap(),
            "dbg_ctxT": nc.dram_tensor("dbg_ctxT", [128, 32], BF16,
                                       kind="ExternalOutput").ap(),
            "dbg_tg0": nc.dram_tensor("dbg_tg0", [128, 128], BF16,
                                      kind="ExternalOutput").ap(),
            "dbg_wep": nc.dram_tensor("dbg_wep", [128, 16 * T * BL], BF16,
                                      kind="ExternalOutput").ap(),
        }

    with tile.TileContext(nc) as tc:
        with (
            tc.tile_pool(name="persist", bufs=1) as pp,
            tc.tile_pool(name="lp", bufs=2) as lp,
            tc.tile_pool(name="pa", bufs=1, space="PSUM") as pa,
            tc.tile_pool(name="pg", bufs=1, space="PSUM") as pg,
        ):
            def dma3(dst, src, n=4):  # dram [n,128,X] -> sbuf [128,n,X]
                for k in range(n):
                    nc.sync.dma_start(out=dst[:, k], in_=src[k])

            # ---- loads ordered by first use (DMA queues drain in order) --
            sp = tc.alloc_tile_pool(name="stream", bufs=1)
            sq = tc.alloc_tile_pool(name="sq", bufs=2, space="PSUM")
            sb_featT = sp.tile([128, 4, BL], BF16, tag="featT")
            dma3(sb_featT, featT)
            sb_wih2 = sp.tile([128, 4, H], BF16, tag="wih2")
            dma3(sb_wih2, wih2)
            sb_wic2 = sp.tile([128, 4, H], BF16, tag="wic2")
            dma3(sb_wic2, wic2)
            sb_bih2 = sp.tile([1, H], BF16, tag="bih2")
            nc.sync.dma_start(out=sb_bih2[:], in_=bih2)
            sb_bic2 = sp.tile([1, H], BF16, tag="bic2")
            nc.sync.dma_start(out=sb_bic2[:], in_=bic2)
            sb_encT = sp.tile([128, 4, BP], BF16, tag="encT")
            dma3(sb_encT, encT)
            sb_wenc = sp.tile([128, 4, A], BF16, tag="wenc")
            dma3(sb_wenc, wenc)
            sb_wdech = pp.tile([128, 4, A], BF16, tag="wdech")
            dma3(sb_wdech, wdech)
            sb_vcol = pp.tile([128, 4, 1], BF16, tag="vcol")
            dma3(sb_vcol, vcol)
            sb_biasadT = pp.tile([128, 4, 1], BF16, tag="biasadT")
            dma3(sb_biasadT, biasadT)
            sb_weT = sp.tile([128, 4, T * BL], BF16, tag="weT")
            dma3(sb_weT, weT)
            sb_wg0x = sp.tile([128, 4, G4], BF16, tag="wg0x")
            dma3(sb_wg0x, wg0x)
            sb_bg0 = sp.tile([1, G4], BF16, tag="bg0")
            nc.sync.dma_start(out=sb_bg0[:], in_=bg0)
            sb_encflat = pp.tile([128, NF, E], BF16, tag="encflat")
            dma3(sb_encflat, encflat, n=NF)
            sb_wg0h = pp.tile([128, 4, G4], BF16, tag="wg0h")
            dma3(sb_wg0h, wg0h)
            sb_wg0c = pp.tile([128, 4, G4], BF16, tag="wg0c")
            dma3(sb_wg0c, wg0c)
            sb_bg1 = pp.tile([1, G4], BF16, tag="bg1")
            nc.sync.dma_start(out=sb_bg1[:], in_=bg1)
            sb_wg1b = pp.tile([128, 4, G4], BF16, tag="wg1b")
            dma3(sb_wg1b, wg1b)
            sb_wg1a = pp.tile([128, 4, G4], BF16, tag="wg1a")
            dma3(sb_wg1a, wg1a)

            i128 = pp.tile([128, 128], BF16, tag="i128")
            make_identity(nc, i128[:])
            ones_1x8 = pp.tile([1, 8], BF16, tag="o18")
            nc.vector.memset(ones_1x8[:], 1.0)
            ones_1xT8 = pp.tile([1, T * BL], BF16, tag="o1T8")
            nc.vector.memset(ones_1xT8[:], 1.0)
            ones_col = pp.tile([128, 1], BF16, tag="ocol")
            nc.vector.memset(ones_col[:], 1.0)
            ones_1x128f = pp.tile([1, 128], F32, tag="o1128f")
            nc.vector.memset(ones_1x128f[:], 1.0)

            # persistent state, two independent 4-batch streams
            # (all transposed: [128, 4, 4] = [dim%128, dim//128, batch-in-stream])
            BS = BL // 2
            C2aT = [pp.tile([128, 4, BS], F32, tag=f"C2aT{s}", name=f"C2aT{s}")
                    for s in (0, 1)]
            C2bT = [pp.tile([128, 4, BS], F32, tag=f"C2bT{s}", name=f"C2bT{s}")
                    for s in (0, 1)]
            H2aT = [pp.tile([128, 4, BS], BF16, tag=f"H2aT{s}", name=f"H2aT{s}")
                    for s in (0, 1)]
            H2b0 = [pp.tile([128, 4, BS], BF16, tag=f"H2b0{s}", name=f"H2b0{s}")
                    for s in (0, 1)]
            hball = pp.tile([128, 4, T * BL], BF16, tag="hball")

            aB = [pp.tile([128, 2 * BS], BF16, tag=f"aB{s}", name=f"aB{s}")
                  for s in (0, 1)]
            nc.vector.memset(aB[0][:], 0.0)
            nc.vector.memset(aB[1][:], 0.0)

            encproj = pp.tile([128, 4, BP], BF16, tag="encproj")
            wepartT = pp.tile([128, 16, T * BL], BF16, tag="wepartT")

            # ---- one-time section ----------------------------------------
            # h0/c0 transposed: out[h%128, hc, b]
            hc0 = sq.tile([128, 512], F32, tag="ot", name="hc0")
            for (w_sb, b_sb, co) in ((sb_wih2, sb_bih2, 0),
                                     (sb_wic2, sb_bic2, 32)):
                for hc in range(4):
                    reg = hc0[:, co + hc * 8:co + (hc + 1) * 8]
                    for k in range(4):
                        nc.tensor.matmul(reg, w_sb[:, k, ts(hc, 128)],
                                         sb_featT[:, k],
                                         start=(k == 0), stop=False,
                                         skip_group_check=True)
                    nc.tensor.matmul(reg, b_sb[:, ts(hc, 128)], ones_1x8[:],
                                     start=False, stop=True,
                                     skip_group_check=True)
            for s in (0, 1):
                h_sl = _sub(hc0[:], [[8, 4], [1, BS]], extra_offset=s * BS)
                c_sl = _sub(hc0[:], [[8, 4], [1, BS]],
                            extra_offset=32 + s * BS)
                nc.scalar.activation(out=H2aT[s][:], in_=h_sl, func=AF.Copy)
                nc.scalar.activation(out=H2b0[s][:], in_=h_sl, func=AF.Copy)
                nc.vector.tensor_copy(C2aT[s][:], c_sl)
                nc.vector.tensor_copy(C2bT[s][:], c_sl)

            # ---- one-time: enc_projT (+ bias) ----------------------------
            QS = BP // 4  # 392
            for k in range(4):          # a-chunk
                for q in range(4):      # (p,b) quarter
                    eps = sq.tile([128, 512], F32, tag="ot", name="eps")
                    for e in range(4):  # e-chunk
                        nc.tensor.matmul(
                            eps[:, 0:QS], sb_wenc[:, e, ts(k, 128)],
                            sb_encT[:, e, ts(q, QS)],
                            start=(e == 0), stop=(e == 3))
                    nc.scalar.activation(out=encproj[:, k, ts(q, QS)],
                                         in_=eps[:, 0:QS], func=AF.Identity,
                                         bias=sb_biasadT[:, k])


            # ================= the recurrent steps ========================
            # Two 4-batch streams per core; emission ping-pongs between them
            # so one stream's big ACT tanh overlaps the other's DVE/PE work.
            def pointwise(nc, lp, gps, s, C2T, h2out, tag):
                """gps = strided [128, 16, BS] view of the gates psum."""
                tg = lp.tile([128, 16, BS], BF16, tag="tg" + tag)
                nc.scalar.activation(
                    out=tg[:].rearrange("p g b -> p (g b)"), in_=gps,
                    func=AF.Tanh)
                tf = _sub(tg[:], [[BS, 4], [1, BS]], extra_offset=0)
                ti = _sub(tg[:], [[BS, 4], [1, BS]], extra_offset=4 * BS)
                tC = _sub(tg[:], [[BS, 4], [1, BS]], extra_offset=8 * BS)
                to = _sub(tg[:], [[BS, 4], [1, BS]], extra_offset=12 * BS)
                s1 = lp.tile([128, 4, BS], F32, tag="s1" + tag)
                nc.vector.scalar_tensor_tensor(
                    out=s1[:], in0=tf, scalar=1.0, in1=C2T[:],
                    op0=ALU.add, op1=ALU.mult)
                s2 = lp.tile([128, 4, BS], F32, tag="s2" + tag)
                nc.vector.scalar_tensor_tensor(
                    out=s2[:], in0=ti, scalar=1.0, in1=tC,
                    op0=ALU.add, op1=ALU.mult)
                nc.vector.scalar_tensor_tensor(
                    out=C2T[:], in0=s1[:], scalar=0.5, in1=s2[:],
                    op0=ALU.mult, op1=ALU.add)
                tch = lp.tile([128, 4, BS], BF16, tag="tch" + tag)
                nc.scalar.activation(out=tch[:], in_=C2T[:],
                                     func=AF.Tanh, scale=0.5)
                nc.vector.scalar_tensor_tensor(
                    out=h2out, in0=to, scalar=1.0, in1=tch[:],
                    op0=ALU.add, op1=ALU.mult)

            # ---- one-time: wepartT = (we @ Wg0x + bg0)^T -----------------
            for gc in range(16):
                wps = sq.tile([128, 512], F32, tag="ot", name="wps")
                nc.tensor.matmul(wps[:, 0:T * BL], sb_bg0[:, ts(gc, 128)],
                                 ones_1xT8[:], start=True, stop=False)
                for k in range(4):
                    nc.tensor.matmul(wps[:, 0:T * BL],
                                     sb_wg0x[:, k, ts(gc, 128)],
                                     sb_weT[:, k],
                                     start=False, stop=(k == 3))
                nc.vector.tensor_copy(wepartT[:, gc], wps[:, 0:T * BL])

            sp.release()
            sq.release()

            # ---- per-op emitters; state dicts keyed by stream ------------
            ST = [{"ps": None, "decT": None, "ssb": None, "rinvr": None,
                   "ctxT": None, "g0": None, "g1": None} for _ in (0, 1)]

            def H2bTk(s, k, t):
                if t == 0:
                    return H2b0[s][:, k]
                return _sub(hball[:], [[1, BS]],
                            extra_offset=k * T * BL + (t - 1) * BL + s * BS)

            def e_dec(s, t):
                st = ST[s]
                st["ps"] = pa.tile([128, 128], F32, tag=f"ps{s}",
                                   name=f"ps{s}")
                for ac in range(4):
                    reg = st["ps"][:, ac * BS:(ac + 1) * BS]
                    for k in range(4):
                        nc.tensor.matmul(reg, sb_wdech[:, k, ts(ac, 128)],
                                         H2bTk(s, k, t),
                                         start=(k == 0), stop=(k == 3),
                                         skip_group_check=True)

            def e_decT(s, k):
                st = ST[s]
                if k != 0:
                    return
                st["decT"] = lp.tile([128, 4, BS], BF16, tag=f"decT{s}",
                                     name=f"decT{s}")
                st["ssb"] = lp.tile([128, 4, P * BS], BF16,
                                    tag=f"ssb{s}", name=f"ssb{s}")
                nc.scalar.activation(
                    out=st["decT"][:].rearrange("p a b -> p (a b)"),
                    in_=st["ps"][:, 0:16], func=AF.Copy)

            def e_add(s, k):
                st = ST[s]
                nc.vector.tensor_tensor(
                    st["ssb"][:, k].rearrange("p (q b) -> p q b", b=BS),
                    _sub(encproj[:, k], [[BL, P], [1, BS]],
                         extra_offset=s * BS),
                    _sub(st["decT"][:], [[0, P], [1, BS]],
                         extra_offset=k * BS),
                    ALU.add)

            def e_tanh(s, k):
                nc.scalar.activation(out=ST[s]["ssb"][:, k],
                                     in_=ST[s]["ssb"][:, k], func=AF.Tanh)

            def e_scores(s):
                st = ST[s]
                for half, pw, poff in ((0, 128, 0), (1, P1, 128 * BS)):
                    for j in range(BS):
                        col = 32 + half * BS + j
                        for k in range(4):
                            nc.tensor.matmul(
                                st["ps"][:pw, col:col + 1],
                                _sub(st["ssb"][:, k], [[BS, pw]],
                                     extra_offset=poff + j),
                                sb_vcol[:, k],
                                start=(k == 0), stop=(k == 3),
                                skip_group_check=True)

            def e_exp(s):
                pst = ST[s]["ps"]
                nc.scalar.activation(out=aB[s][:, 0:BS],
                                     in_=pst[:, 32:32 + BS], func=AF.Exp)
                nc.scalar.activation(out=aB[s][0:P1, BS:2 * BS],
                                     in_=pst[0:P1, 32 + BS:32 + 2 * BS],
                                     func=AF.Exp)

            def e_sums(s):
                pst = ST[s]["ps"]
                nc.tensor.matmul(pst[0:1, 48:48 + BS], ones_col[:],
                                 aB[s][:, 0:BS], start=True, stop=False,
                                 skip_group_check=True)
                nc.tensor.matmul(pst[0:1, 48:48 + BS], ones_col[:],
                                 aB[s][:, BS:2 * BS], start=False,
                                 stop=True, skip_group_check=True)

            def e_recip(s):
                st = ST[s]
                st["rinvr"] = lp.tile([1, BS], F32, tag=f"rinvr{s}",
                                      name=f"rinvr{s}")
                nc.vector.reciprocal(st["rinvr"][:],
                                     st["ps"][0:1, 48:48 + BS])

            def e_ctx(s):
                pst = ST[s]["ps"]
                for j in range(BS):
                    b = s * BS + j
                    for ec in range(4):
                        col = 64 + ec * BS + j
                        for half in (0, 1):
                            nc.tensor.matmul(
                                pst[:, col:col + 1],
                                sb_encflat[:, 2 * b + half, ts(ec, 128)],
                                aB[s][:, half * BS + j:half * BS + j + 1],
                                start=(half == 0), stop=(half == 1),
                                skip_group_check=True)

            def e_ctxT(s):
                st = ST[s]
                rb = lp.tile([128, BS], F32, tag=f"rb{s}", name=f"rb{s}")
                nc.gpsimd.partition_broadcast(rb[:], st["rinvr"][:])
                st["ctxT"] = lp.tile([128, 4, BS], BF16, tag=f"ctxT{s}",
                                     name=f"ctxT{s}")
                nc.vector.tensor_tensor(
                    st["ctxT"][:],
                    _sub(st["ps"][:], [[BS, 4], [1, BS]], extra_offset=64),
                    _sub(rb[:], [[0, 4], [1, BS]]),
                    ALU.mult)

            def e_g0(s, t):
                st = ST[s]
                st["g0"] = pg.tile([128, 64], F32, tag=f"g0{s}",
                                   name=f"g0{s}")
                for gc in range(16):
                    reg = st["g0"][:, gc * BS:(gc + 1) * BS]
                    nc.tensor.matmul(
                        reg, i128[:],
                        wepartT[:, gc,
                                t * BL + s * BS:t * BL + (s + 1) * BS],
                        start=True, stop=False)
                    for k in range(4):
                        nc.tensor.matmul(reg, sb_wg0h[:, k, ts(gc, 128)],
                                         H2aT[s][:, k], start=False,
                                         stop=False)
                    for k in range(4):
                        nc.tensor.matmul(reg, sb_wg0c[:, k, ts(gc, 128)],
                                         st["ctxT"][:, k], start=False,
                                         stop=(k == 3))

            def e_pw0(s):
                pointwise(nc, lp, _sub(ST[s]["g0"][:], [[BS, 16], [1, BS]]),
                          s, C2aT[s], H2aT[s][:], f"0{s}")

            def e_g1(s, t):
                st = ST[s]
                st["g1"] = pg.tile([128, 64], F32, tag=f"g1{s}",
                                   name=f"g1{s}")
                for gc in range(16):
                    reg = st["g1"][:, gc * BS:(gc + 1) * BS]
                    nc.tensor.matmul(reg, sb_bg1[:, ts(gc, 128)],
                                     ones_1x8[:, 0:BS],
                                     start=True, stop=False)
                    for k in range(4):
                        nc.tensor.matmul(reg, sb_wg1b[:, k, ts(gc, 128)],
                                         H2bTk(s, k, t), start=False,
                                         stop=False)
                    for k in range(4):
                        nc.tensor.matmul(reg, sb_wg1a[:, k, ts(gc, 128)],
                                         H2aT[s][:, k], start=False,
                                         stop=(k == 3))

            def e_pw1(s, t):
                h2b_out = _sub(hball[:], [[T * BL, 4], [1, BS]],
                               extra_offset=t * BL + s * BS)
                pointwise(nc, lp, _sub(ST[s]["g1"][:], [[BS, 16], [1, BS]]),
                          s, C2bT[s], h2b_out, f"1{s}")

            def front(s, t):
                """dec + adds + tanh + scores, self-contained (prologue)."""
                e_dec(s, t)
                for k in range(4):
                    e_decT(s, k)
                    e_add(s, k)
                    e_tanh(s, k)
                e_scores(s)

            def slot(sb_, sf, t_back, t_front, do_front):
                """Emit stream sb_'s back-half of step t_back, micro-
                interleaved with stream sf's front-half of step t_front."""
                if do_front:
                    e_dec(sf, t_front)
                e_exp(sb_)
                if do_front:
                    e_decT(sf, 0)
                    e_add(sf, 0)
                e_sums(sb_)
                e_recip(sb_)
                if do_front:
                    e_tanh(sf, 0)
                    e_decT(sf, 1)
                    e_add(sf, 1)
                e_ctx(sb_)
                e_ctxT(sb_)
                if do_front:
                    e_tanh(sf, 1)
                    e_decT(sf, 2)
                e_g0(sb_, t_back)
                if do_front:
                    e_add(sf, 2)
                e_pw0(sb_)
                if do_front:
                    e_tanh(sf, 2)
                    e_decT(sf, 3)
                    e_add(sf, 3)
                e_g1(sb_, t_back)
                if do_front:
                    e_tanh(sf, 3)
                e_pw1(sb_, t_back)
                if do_front:
                    e_scores(sf)

            # ---- software-pipelined schedule: stream 1 lags half a step --
            front(0, 0)
            for t in range(n_steps):
                slot(0, 1, t, t, True)                       # s0 back | s1 front
                slot(1, 0, t, t + 1, t + 1 < n_steps)        # s1 back | s0 front

            for k in range(4):
                nc.sync.dma_start(out=hballT[k], in_=hball[:, k])

    nc.compile()
    return nc


# --------------------------------------------------------------------------
# Phase B module: logitsT = (0.5*W_out)^T @ H2b_all + b_out  (vocab-sharded)
# --------------------------------------------------------------------------

def build_phase_b():
    nc = bacc.Bacc("TRN2", num_devices=NC, debug=False)
    hbT = nc.dram_tensor("hbT", [4, 128, BT], BF16, kind="ExternalInput").ap()
    wout = nc.dram_tensor("wout", [4, 128, VSL], BF16,
                          kind="ExternalInput").ap()
    boutT = nc.dram_tensor("boutT", [128, 30], F32,
                           kind="ExternalInput").ap()
    logits = nc.dram_tensor("logits", [VSL, BT], BF16,
                            kind="ExternalOutput").ap()

    vtiles = [(vt, vt * 128, min(128, VSL - vt * 128)) for vt in range(30)]
    nchunks = [(0, 512), (512, 512), (1024, 256)]

    with tile.TileContext(nc) as tc:
        with (
            tc.tile_pool(name="w", bufs=1) as wp,
            tc.tile_pool(name="l", bufs=4) as lp,
            tc.tile_pool(name="ps", bufs=2, space="PSUM") as ps,
        ):
            sb_wout = wp.tile([128, 4, VSL], BF16, tag="wout")
            for k in range(4):
                nc.sync.dma_start(out=sb_wout[:, k, 0:128],
                                  in_=wout[k, :, 0:128])
            sb_hbT = wp.tile([128, 4, BT], BF16, tag="hbT")
            for k in range(4):
                for h in range(2):
                    nc.sync.dma_start(out=sb_hbT[:, k, ts(h, BT // 2)],
                                      in_=hbT[k, :, ts(h, BT // 2)])
            sb_boutT = wp.tile([128, 30], F32, tag="boutT")
            nc.sync.dma_start(out=sb_boutT[:], in_=boutT)
            vg = [(128, 832), (960, 960), (1920, 960), (2880, 870)]
            for g0, gw in vg:
                for k in range(4):
                    nc.sync.dma_start(out=sb_wout[:, k, g0:g0 + gw],
                                      in_=wout[k, :, g0:g0 + gw])

            for vt, v0, vw in vtiles:
                pt = ps.tile([128, BT], F32, tag="acc")
                for n0, nw in nchunks:
                    for k in range(4):
                        nc.tensor.matmul(pt[:vw, n0:n0 + nw],
                                         sb_wout[:, k, v0:v0 + vw],
                                         sb_hbT[:, k, n0:n0 + nw],
                                         start=(k == 0), stop=(k == 3),
                                         skip_group_check=True)
                ot = lp.tile([128, BT], BF16, tag="out")
                if vt % 2 == 0:
                    nc.scalar.activation(out=ot[:vw], in_=pt[:vw],
                                         func=AF.Identity,
                                         bias=sb_boutT[:vw, vt:vt + 1])
                else:
                    nc.vector.tensor_scalar(
                        out=ot[:vw], in0=pt[:vw],
                        scalar1=sb_boutT[:vw, vt:vt + 1], scalar2=None,
                        op0=ALU.add)
                nc.sync.dma_start(out=logits[v0:v0 + vw], in_=ot[:vw])
    nc.compile()
    return nc


# --------------------------------------------------------------------------
# Host-side preparation + driver
# --------------------------------------------------------------------------

def prep_phase_a_inputs(features, encoder_out, emb, W_enc, b_enc, W_dec, b_dec,
                        v_w, W_g0, b_g0, W_g1, b_g1, W_ih, b_ih, W_ic, b_ic,
                        captions):
    embeds = np.asarray(emb)[np.asarray(captions)[:, :T].astype(np.int64)]
    cs = np.ones((G4,), np.float32)     # sigmoid halving on f, i, o columns
    cs[0:H] = 0.5
    cs[H:2 * H] = 0.5
    cs[3 * H:4 * H] = 0.5
    W_g0 = np.asarray(W_g0) * cs
    W_g1 = np.asarray(W_g1) * cs
    shared = {
        "wih2": _bf16(2.0 * np.asarray(W_ih).reshape(4, 128, H)),
        "wic2": _bf16(2.0 * np.asarray(W_ic).reshape(4, 128, H)),
        "bih2": _bf16(2.0 * np.asarray(b_ih).reshape(1, H)),
        "bic2": _bf16(2.0 * np.asarray(b_ic).reshape(1, H)),
        "wenc": _bf16(np.asarray(W_enc).reshape(4, 128, A)),
        "biasadT": _bf16((np.asarray(b_enc) + np.asarray(b_dec))
                         .reshape(4, 128, 1)),
        "wdech": _bf16(0.5 * np.asarray(W_dec).reshape(4, 128, A)),
        "vcol": _bf16(np.asarray(v_w).reshape(4, 128, 1)),
        "wg0x": _bf16(W_g0[:E].reshape(4, 128, G4)),
        "bg0": _bf16((np.asarray(b_g0) * cs).reshape(1, G4)),
        "wg0c": _bf16(W_g0[E:2 * E].reshape(4, 128, G4)),
        "wg0h": _bf16(0.5 * W_g0[2 * E:].reshape(4, 128, G4)),
        "wg1a": _bf16(0.5 * W_g1[:H].reshape(4, 128, G4)),
        "wg1b": _bf16(0.5 * W_g1[H:].reshape(4, 128, G4)),
        "bg1": _bf16((np.asarray(b_g1) * cs).reshape(1, G4)),
    }
    in_maps = []
    for c in range(NC):
        bs = slice(c * BL, (c + 1) * BL)
        enc = np.asarray(encoder_out)[bs]               # [8, 196, 512]
        encTn = enc.transpose(2, 1, 0).reshape(E, BP)   # (e, p, b)
        encpad = np.zeros((BL, PPAD, E), np.float32)
        encpad[:, :P] = enc
        feat = np.asarray(features)[bs]
        we = embeds[bs]                                 # [8, T, E]
        m = dict(shared)
        m["encT"] = _bf16(encTn.reshape(4, 128, BP))
        m["encflat"] = _bf16(encpad.reshape(NF, 128, E))
        m["featT"] = _bf16(feat.T.reshape(4, 128, BL))
        m["weT"] = _bf16(we.transpose(2, 1, 0).reshape(4, 128, T * BL))
        in_maps.append(m)
    return in_maps


_CACHE = {}


def kernel(**inputs):
    inputs = {k: np.asarray(v) for k, v in inputs.items()}
    if "a" not in _CACHE:
        _CACHE["a"] = build_phase_a()
    if "b" not in _CACHE:
        _CACHE["b"] = build_phase_b()

    in_a = prep_phase_a_inputs(
        inputs["features"], inputs["encoder_out"], inputs["emb"],
        inputs["W_enc"], inputs["b_enc"], inputs["W_dec"], inputs["b_dec"],
        inputs["v_w"], inputs["W_g0"], inputs["b_g0"], inputs["W_g1"],
        inputs["b_g1"], inputs["W_ih"], inputs["b_ih"], inputs["W_ic"],
        inputs["b_ic"], inputs["captions"])
    ra = run_bass_kernel_spmd(_CACHE["a"], in_a, core_ids=list(range(NC)))

    # reassemble hb: column index b*T + t
    hbT_full = np.zeros((4, 128, BT), dtype=ml_dtypes.bfloat16)
    for c in range(NC):
        part = ra.results[c]["hballT"].reshape(4, 128, T, BL)
        for bl in range(BL):
            b = c * BL + bl
            hbT_full[:, :, b * T:(b + 1) * T] = part[:, :, :, bl]

    W_out = np.asarray(inputs["W_out"])
    b_out = np.asarray(inputs["b_out"])
    bpad = np.zeros((30 * 128,), np.float32)
    in_b = []
    for c in range(NC):
        vs = slice(c * VSL, (c + 1) * VSL)
        bpad[:VSL] = b_out[vs]
        in_b.append({
            "hbT": hbT_full,
            "wout": _bf16(0.5 * W_out[:, vs].reshape(4, 128, VSL)),
            "boutT": np.ascontiguousarray(bpad.reshape(30, 128).T,
                                          dtype=np.float32),
        })
    rb = run_bass_kernel_spmd(_CACHE["b"], in_b, core_ids=list(range(NC)))
    out = np.empty((BT, V), np.float32)
    for c in range(NC):
        vs = slice(c * VSL, (c + 1) * VSL)
        out[:, vs] = rb.results[c]["logits"].astype(np.float32).T
    return out.reshape(B, T, V)


# revision 48
# speedup vs baseline: 1.0581x; 1.0000x over previous
"""Trainium2 Bass kernel for nn_CaptionDecoder (attention LSTM caption decoder).

Strategy (8 NeuronCores):
  Phase A: data-parallel over batch (8 batches/core) for the sequential
           attention + 2-layer-LSTM recurrence. Every per-step matmul is in
           transposed orientation (feature dim on PSUM partitions, batch as
           the streamed free dim), so gates, dec_proj, attention scores and
           context all come out of PSUM already transposed and the LSTM
           pointwise runs on 128-partition tiles; there are no per-step DMAs
           and no per-step PE transposes. The 8 batches are split into two
           independent 4-batch streams, software-pipelined with a half-step
           skew so one stream's attention tanh (the scalar-engine floor)
           overlaps the other stream's matmul/vector segments.
  Host:    gathers top-layer hidden states hb from the 8 cores (1.3 MB).
  Phase B: vocab-parallel logits projection in transposed layout
           (vocab tile on partitions, all 1280 (b,t) rows streamed), so
           b_out becomes a per-partition bias folded into the PSUM
           evacuation, which alternates between ACT and DVE to stay off
           the tensor-engine critical path.

Precision: bf16 matmuls with fp32 PSUM accumulation; fp32 cell state.
Sigmoid is computed as 0.5*(1+tanh(x/2)); the 0.5 factors are folded into
host-prescaled weights (column scale on f,i,o gates) and doubled states
H2=2h, C2=2c (row scale on recurrent weights), so one tanh over all 2048
gate pre-activations covers every gate nonlinearity.
"""

import numpy as np
import ml_dtypes

import concourse.bass as bass
import concourse.bacc as bacc
import concourse.mybir as mybir
import concourse.tile as tile
from concourse.bass import ts
from concourse.bass_utils import run_bass_kernel_spmd
from concourse.masks import make_identity

F32 = mybir.dt.float32
BF16 = mybir.dt.bfloat16
AF = mybir.ActivationFunctionType
ALU = mybir.AluOpType

B, TC, P, E, H, A, V = 64, 21, 196, 512, 512, 512, 30000
T = TC - 1            # 20 decode steps
NC = 8                # cores
BL = B // NC          # 8 batches per core
PPAD = 256            # padded attention positions per batch
NF = BL * PPAD // 128  # 16 (batch, p-half) chunks
BP = BL * P           # 1568 (p, b) columns per core
G4 = 4 * H            # 2048 stacked gates f,i,C,o
VSL = V // NC         # 3750 vocab rows per core (phase B)
BT = B * T            # 1280 (b, t) columns
P1 = P - 128          # 68 positions in the second p-half


def _bf16(x):
    return np.ascontiguousarray(np.asarray(x, dtype=np.float32)).astype(
        ml_dtypes.bfloat16)


def _sub(ap, dims, extra_offset=0):
    """Custom free-dim access pattern on an AP, keeping its partition dim."""
    return bass.AP(ap.tensor, ap.offset + extra_offset,
                   [list(ap.ap[0])] + [list(d) for d in dims])


def _pbcast(ap, dims, extra_offset=0):
    """Partition-broadcast (stride 0) custom AP."""
    return bass.AP(ap.tensor, ap.offset + extra_offset,
                   [[0, 128]] + [list(d) for d in dims])


# --------------------------------------------------------------------------
# Phase A module: the recurrence
# --------------------------------------------------------------------------

DBG = False


def build_phase_a(n_steps=T):
    nc = bacc.Bacc("TRN2", num_devices=NC, debug=False)

    def din(name, shape, dt=BF16):
        return nc.dram_tensor(name, shape, dt, kind="ExternalInput").ap()

    encT = din("encT", [4, 128, BP])          # encoder_out^T  [a-chk][a][(p,b)]
    encflat = din("encflat", [NF, 128, E])    # [(b,half)][p][e], 0-padded
    featT = din("featT", [4, 128, BL])
    wih2 = din("wih2", [4, 128, H])           # 2*W_ih
    wic2 = din("wic2", [4, 128, H])           # 2*W_ic
    bih2 = din("bih2", [1, H])
    bic2 = din("bic2", [1, H])
    wenc = din("wenc", [4, 128, A])
    biasadT = din("biasadT", [4, 128, 1])     # b_enc + b_dec, transposed
    wdech = din("wdech", [4, 128, A])         # 0.5*W_dec
    vcol = din("vcol", [4, 128, 1])
    weT = din("weT", [4, 128, T * BL])        # embeds^T, col = t*8+b
    wg0x = din("wg0x", [4, 128, G4])          # col-scaled
    bg0 = din("bg0", [1, G4])
    wg0c = din("wg0c", [4, 128, G4])
    wg0h = din("wg0h", [4, 128, G4])
    wg1a = din("wg1a", [4, 128, G4])
    wg1b = din("wg1b", [4, 128, G4])
    bg1 = din("bg1", [1, G4])

    hballT = nc.dram_tensor("hballT", [4, 128, T * BL], BF16,
                            kind="ExternalOutput").ap()
    if DBG:
        dbg = {
            "dbg_h0": nc.dram_tensor("dbg_h0", [128, 32], BF16,
                                     kind="ExternalOutput").ap(),
            "dbg_c0": nc.dram_tensor("dbg_c0", [128, 32], F32,
                                     kind="ExternalOutput").ap(),
            "dbg_encproj": nc.dram_tensor("dbg_encproj", [128, 4 * BP], BF16,
                                          kind="ExternalOutput").ap(),
            "dbg_decT": nc.dram_tensor("dbg_decT", [128, 32], BF16,
                                       kind="ExternalOutput").ap(),
            "dbg_ssb": nc.dram_tensor("dbg_ssb", [128, 4 * BP], BF16,
                                      kind="ExternalOutput").ap(),
            "dbg_aB": nc.dram_tensor("dbg_aB", [128, 16], BF16,
                                     kind="ExternalOutput").ap(),
            "dbg_ctxT": nc.dram_tensor("dbg_ctxT", [128, 32], BF16,
                                       kind="ExternalOutput").ap(),
            "dbg_tg0": nc.dram_tensor("dbg_tg0", [128, 128], BF16,
                                      kind="ExternalOutput").ap(),
            "dbg_wep": nc.dram_tensor("dbg_wep", [128, 16 * T * BL], BF16,
                                      kind="ExternalOutput").ap(),
        }

    with tile.TileContext(nc) as tc:
        with (
            tc.tile_pool(name="persist", bufs=1) as pp,
            tc.tile_pool(name="lp", bufs=2) as lp,
            tc.tile_pool(name="pa", bufs=1, space="PSUM") as pa,
            tc.tile_pool(name="pg", bufs=1, space="PSUM") as pg,
        ):
            def dma3(dst, src, n=4):  # dram [n,128,X] -> sbuf [128,n,X]
                for k in range(n):
                    nc.sync.dma_start(out=dst[:, k], in_=src[k])

            # ---- loads ordered by first use (DMA queues drain in order) --
            sp = tc.alloc_tile_pool(name="stream", bufs=1)
            sq = tc.alloc_tile_pool(name="sq", bufs=2, space="PSUM")
            sb_featT = sp.tile([128, 4, BL], BF16, tag="featT")
            dma3(sb_featT, featT)
            sb_wih2 = sp.tile([128, 4, H], BF16, tag="wih2")
            dma3(sb_wih2, wih2)
            sb_wic2 = sp.tile([128, 4, H], BF16, tag="wic2")
            dma3(sb_wic2, wic2)
            sb_bih2 = sp.tile([1, H], BF16, tag="bih2")
            nc.sync.dma_start(out=sb_bih2[:], in_=bih2)
            sb_bic2 = sp.tile([1, H], BF16, tag="bic2")
            nc.sync.dma_start(out=sb_bic2[:], in_=bic2)
            sb_encT = sp.tile([128, 4, BP], BF16, tag="encT")
            dma3(sb_encT, encT)
            sb_wenc = sp.tile([128, 4, A], BF16, tag="wenc")
            dma3(sb_wenc, wenc)
            sb_wdech = pp.tile([128, 4, A], BF16, tag="wdech")
            dma3(sb_wdech, wdech)
            sb_vcol = pp.tile([128, 4, 1], BF16, tag="vcol")
            dma3(sb_vcol, vcol)
            sb_biasadT = pp.tile([128, 4, 1], BF16, tag="biasadT")
            dma3(sb_biasadT, biasadT)
            sb_weT = sp.tile([128, 4, T * BL], BF16, tag="weT")
            dma3(sb_weT, weT)
            sb_wg0x = sp.tile([128, 4, G4], BF16, tag="wg0x")
            dma3(sb_wg0x, wg0x)
            sb_bg0 = sp.tile([1, G4], BF16, tag="bg0")
            nc.sync.dma_start(out=sb_bg0[:], in_=bg0)
            sb_encflat = pp.tile([128, NF, E], BF16, tag="encflat")
            dma3(sb_encflat, encflat, n=NF)
            sb_wg0h = pp.tile([128, 4, G4], BF16, tag="wg0h")
            dma3(sb_wg0h, wg0h)
            sb_wg0c = pp.tile([128, 4, G4], BF16, tag="wg0c")
            dma3(sb_wg0c, wg0c)
            sb_bg1 = pp.tile([1, G4], BF16, tag="bg1")
            nc.sync.dma_start(out=sb_bg1[:], in_=bg1)
            sb_wg1b = pp.tile([128, 4, G4], BF16, tag="wg1b")
            dma3(sb_wg1b, wg1b)
            sb_wg1a = pp.tile([128, 4, G4], BF16, tag="wg1a")
            dma3(sb_wg1a, wg1a)

            i128 = pp.tile([128, 128], BF16, tag="i128")
            make_identity(nc, i128[:])
            ones_1x8 = pp.tile([1, 8], BF16, tag="o18")
            nc.vector.memset(ones_1x8[:], 1.0)
            ones_1xT8 = pp.tile([1, T * BL], BF16, tag="o1T8")
            nc.vector.memset(ones_1xT8[:], 1.0)
            ones_col = pp.tile([128, 1], BF16, tag="ocol")
            nc.vector.memset(ones_col[:], 1.0)
            ones_1x128f = pp.tile([1, 128], F32, tag="o1128f")
            nc.vector.memset(ones_1x128f[:], 1.0)

            # persistent state, two independent 4-batch streams
            # (all transposed: [128, 4, 4] = [dim%128, dim//128, batch-in-stream])
            BS = BL // 2
            C2aT = [pp.tile([128, 4, BS], F32, tag=f"C2aT{s}", name=f"C2aT{s}")
                    for s in (0, 1)]
            C2bT = [pp.tile([128, 4, BS], F32, tag=f"C2bT{s}", name=f"C2bT{s}")
                    for s in (0, 1)]
            H2aT = [pp.tile([128, 4, BS], BF16, tag=f"H2aT{s}", name=f"H2aT{s}")
                    for s in (0, 1)]
            H2b0 = [pp.tile([128, 4, BS], BF16, tag=f"H2b0{s}", name=f"H2b0{s}")
                    for s in (0, 1)]
            hball = pp.tile([128, 4, T * BL], BF16, tag="hball")

            aB = [pp.tile([128, 2 * BS], BF16, tag=f"aB{s}", name=f"aB{s}")
                  for s in (0, 1)]
            nc.vector.memset(aB[0][:], 0.0)
            nc.vector.memset(aB[1][:], 0.0)

            encproj = pp.tile([128, 4, BP], BF16, tag="encproj")
            wepartT = pp.tile([128, 16, T * BL], BF16, tag="wepartT")

            # ---- one-time section ----------------------------------------
            # h0/c0 transposed: out[h%128, hc, b]
            hc0 = sq.tile([128, 512], F32, tag="ot", name="hc0")
            for (w_sb, b_sb, co) in ((sb_wih2, sb_bih2, 0),
                                     (sb_wic2, sb_bic2, 32)):
                for hc in range(4):
                    reg = hc0[:, co + hc * 8:co + (hc + 1) * 8]
                    for k in range(4):
                        nc.tensor.matmul(reg, w_sb[:, k, ts(hc, 128)],
                                         sb_featT[:, k],
                                         start=(k == 0), stop=False,
                                         skip_group_check=True)
                    nc.tensor.matmul(reg, b_sb[:, ts(hc, 128)], ones_1x8[:],
                                     start=False, stop=True,
                                     skip_group_check=True)
            for s in (0, 1):
                h_sl = _sub(hc0[:], [[8, 4], [1, BS]], extra_offset=s * BS)
                c_sl = _sub(hc0[:], [[8, 4], [1, BS]],
                            extra_offset=32 + s * BS)
                nc.scalar.activation(out=H2aT[s][:], in_=h_sl, func=AF.Copy)
                nc.scalar.activation(out=H2b0[s][:], in_=h_sl, func=AF.Copy)
                nc.vector.tensor_copy(C2aT[s][:], c_sl)
                nc.vector.tensor_copy(C2bT[s][:], c_sl)

            # ---- one-time: enc_projT (+ bias) ----------------------------
            QS = BP // 4  # 392
            for k in range(4):          # a-chunk
                for q in range(4):      # (p,b) quarter
                    eps = sq.tile([128, 512], F32, tag="ot", name="eps")
                    for e in range(4):  # e-chunk
                        nc.tensor.matmul(
                            eps[:, 0:QS], sb_wenc[:, e, ts(k, 128)],
                            sb_encT[:, e, ts(q, QS)],
                            start=(e == 0), stop=(e == 3))
                    nc.scalar.activation(out=encproj[:, k, ts(q, QS)],
                                         in_=eps[:, 0:QS], func=AF.Identity,
                                         bias=sb_biasadT[:, k])


            # ================= the recurrent steps ========================
            # Two 4-batch streams per core; emission ping-pongs between them
            # so one stream's big ACT tanh overlaps the other's DVE/PE work.
            def pointwise(nc, lp, gps, s, C2T, h2out, tag):
                """gps = strided [128, 16, BS] view of the gates psum."""
                tg = lp.tile([128, 16, BS], BF16, tag="tg" + tag)
                nc.scalar.activation(
                    out=tg[:].rearrange("p g b -> p (g b)"), in_=gps,
                    func=AF.Tanh)
                tf = _sub(tg[:], [[BS, 4], [1, BS]], extra_offset=0)
                ti = _sub(tg[:], [[BS, 4], [1, BS]], extra_offset=4 * BS)
                tC = _sub(tg[:], [[BS, 4], [1, BS]], extra_offset=8 * BS)
                to = _sub(tg[:], [[BS, 4], [1, BS]], extra_offset=12 * BS)
                s1 = lp.tile([128, 4, BS], F32, tag="s1" + tag)
                nc.vector.scalar_tensor_tensor(
                    out=s1[:], in0=tf, scalar=1.0, in1=C2T[:],
                    op0=ALU.add, op1=ALU.mult)
                s2 = lp.tile([128, 4, BS], F32, tag="s2" + tag)
                nc.vector.scalar_tensor_tensor(
                    out=s2[:], in0=ti, scalar=1.0, in1=tC,
                    op0=ALU.add, op1=ALU.mult)
                nc.vector.scalar_tensor_tensor(
                    out=C2T[:], in0=s1[:], scalar=0.5, in1=s2[:],
                    op0=ALU.mult, op1=ALU.add)
                tch = lp.tile([128, 4, BS], BF16, tag="tch" + tag)
                nc.scalar.activation(out=tch[:], in_=C2T[:],
                                     func=AF.Tanh, scale=0.5)
                nc.vector.scalar_tensor_tensor(
                    out=h2out, in0=to, scalar=1.0, in1=tch[:],
                    op0=ALU.add, op1=ALU.mult)

            # ---- one-time: wepartT = (we @ Wg0x + bg0)^T -----------------
            for gc in range(16):
                wps = sq.tile([128, 512], F32, tag="ot", name="wps")
                nc.tensor.matmul(wps[:, 0:T * BL], sb_bg0[:, ts(gc, 128)],
                                 ones_1xT8[:], start=True, stop=False)
                for k in range(4):
                    nc.tensor.matmul(wps[:, 0:T * BL],
                                     sb_wg0x[:, k, ts(gc, 128)],
                                     sb_weT[:, k],
                                     start=False, stop=(k == 3))
                nc.vector.tensor_copy(wepartT[:, gc], wps[:, 0:T * BL])

            sp.release()
            sq.release()

            # ---- per-op emitters; state dicts keyed by stream ------------
            ST = [{"ps": None, "decT": None, "ssb": None, "rinvr": None,
                   "ctxT": None, "g0": None, "g1": None} for _ in (0, 1)]

            def H2bTk(s, k, t):
                if t == 0:
                    return H2b0[s][:, k]
                return _sub(hball[:], [[1, BS]],
                            extra_offset=k * T * BL + (t - 1) * BL + s * BS)

            def e_dec(s, t):
                st = ST[s]
                st["ps"] = pa.tile([128, 128], F32, tag=f"ps{s}",
                                   name=f"ps{s}")
                for ac in range(4):
                    reg = st["ps"][:, ac * BS:(ac + 1) * BS]
                    for k in range(4):
                        nc.tensor.matmul(reg, sb_wdech[:, k, ts(ac, 128)],
                                         H2bTk(s, k, t),
                                         start=(k == 0), stop=(k == 3),
                                         skip_group_check=True)

            def e_decT(s, k):
                st = ST[s]
                if k != 0:
                    return
                st["decT"] = lp.tile([128, 4, BS], BF16, tag=f"decT{s}",
                                     name=f"decT{s}")
                st["ssb"] = lp.tile([128, 4, P * BS], BF16,
                                    tag=f"ssb{s}", name=f"ssb{s}")
                nc.scalar.activation(
                    out=st["decT"][:].rearrange("p a b -> p (a b)"),
                    in_=st["ps"][:, 0:16], func=AF.Copy)

            def e_add(s, k):
                st = ST[s]
                nc.vector.tensor_tensor(
                    st["ssb"][:, k].rearrange("p (q b) -> p q b", b=BS),
                    _sub(encproj[:, k], [[BL, P], [1, BS]],
                         extra_offset=s * BS),
                    _sub(st["decT"][:], [[0, P], [1, BS]],
                         extra_offset=k * BS),
                    ALU.add)

            def e_tanh(s, kp):
                ap = _sub(ST[s]["ssb"][:], [[1, 2 * P * BS]],
                          extra_offset=kp * 2 * P * BS)
                nc.scalar.activation(out=ap, in_=ap, func=AF.Tanh)

            def e_scores(s):
                st = ST[s]
                for half, pw, poff in ((0, 128, 0), (1, P1, 128 * BS)):
                    for j in range(BS):
                        col = 32 + half * BS + j
                        for k in range(4):
                            nc.tensor.matmul(
                                st["ps"][:pw, col:col + 1],
                                _sub(st["ssb"][:, k], [[BS, pw]],
                                     extra_offset=poff + j),
                                sb_vcol[:, k],
                                start=(k == 0), stop=(k == 3),
                                skip_group_check=True)

            def e_exp(s):
                pst = ST[s]["ps"]
                nc.scalar.activation(out=aB[s][:, 0:BS],
                                     in_=pst[:, 32:32 + BS], func=AF.Exp)
                nc.scalar.activation(out=aB[s][0:P1, BS:2 * BS],
                                     in_=pst[0:P1, 32 + BS:32 + 2 * BS],
                                     func=AF.Exp)

            def e_sums(s):
                pst = ST[s]["ps"]
                nc.tensor.matmul(pst[0:1, 48:48 + BS], ones_col[:],
                                 aB[s][:, 0:BS], start=True, stop=False,
                                 skip_group_check=True)
                nc.tensor.matmul(pst[0:1, 48:48 + BS], ones_col[:],
                                 aB[s][:, BS:2 * BS], start=False,
                                 stop=True, skip_group_check=True)

            def e_recip(s):
                st = ST[s]
                st["rinvr"] = lp.tile([1, BS], F32, tag=f"rinvr{s}",
                                      name=f"rinvr{s}")
                nc.vector.reciprocal(st["rinvr"][:],
                                     st["ps"][0:1, 48:48 + BS])

            def e_ctx(s):
                pst = ST[s]["ps"]
                for j in range(BS):
                    b = s * BS + j
                    for ec in range(4):
                        col = 64 + ec * BS + j
                        for half in (0, 1):
                            nc.tensor.matmul(
                                pst[:, col:col + 1],
                                sb_encflat[:, 2 * b + half, ts(ec, 128)],
                                aB[s][:, half * BS + j:half * BS + j + 1],
                                start=(half == 0), stop=(half == 1),
                                skip_group_check=True)

            def e_ctxT(s):
                st = ST[s]
                rb = lp.tile([128, BS], F32, tag=f"rb{s}", name=f"rb{s}")
                nc.gpsimd.partition_broadcast(rb[:], st["rinvr"][:])
                st["ctxT"] = lp.tile([128, 4, BS], BF16, tag=f"ctxT{s}",
                                     name=f"ctxT{s}")
                nc.vector.tensor_tensor(
                    st["ctxT"][:],
                    _sub(st["ps"][:], [[BS, 4], [1, BS]], extra_offset=64),
                    _sub(rb[:], [[0, 4], [1, BS]]),
                    ALU.mult)

            def e_g0(s, t):
                st = ST[s]
                st["g0"] = pg.tile([128, 64], F32, tag=f"g0{s}",
                                   name=f"g0{s}")
                for gc in range(16):
                    reg = st["g0"][:, gc * BS:(gc + 1) * BS]
                    nc.tensor.matmul(
                        reg, i128[:],
                        wepartT[:, gc,
                                t * BL + s * BS:t * BL + (s + 1) * BS],
                        start=True, stop=False)
                    for k in range(4):
                        nc.tensor.matmul(reg, sb_wg0h[:, k, ts(gc, 128)],
                                         H2aT[s][:, k], start=False,
                                         stop=False)
                    for k in range(4):
                        nc.tensor.matmul(reg, sb_wg0c[:, k, ts(gc, 128)],
                                         st["ctxT"][:, k], start=False,
                                         stop=(k == 3))

            def e_pw0(s):
                pointwise(nc, lp, _sub(ST[s]["g0"][:], [[BS, 16], [1, BS]]),
                          s, C2aT[s], H2aT[s][:], f"0{s}")

            def e_g1(s, t):
                st = ST[s]
                st["g1"] = pg.tile([128, 64], F32, tag=f"g1{s}",
                                   name=f"g1{s}")
                for gc in range(16):
                    reg = st["g1"][:, gc * BS:(gc + 1) * BS]
                    nc.tensor.matmul(reg, sb_bg1[:, ts(gc, 128)],
                                     ones_1x8[:, 0:BS],
                                     start=True, stop=False)
                    for k in range(4):
                        nc.tensor.matmul(reg, sb_wg1b[:, k, ts(gc, 128)],
                                         H2bTk(s, k, t), start=False,
                                         stop=False)
                    for k in range(4):
                        nc.tensor.matmul(reg, sb_wg1a[:, k, ts(gc, 128)],
                                         H2aT[s][:, k], start=False,
                                         stop=(k == 3))

            def e_pw1(s, t):
                h2b_out = _sub(hball[:], [[T * BL, 4], [1, BS]],
                               extra_offset=t * BL + s * BS)
                pointwise(nc, lp, _sub(ST[s]["g1"][:], [[BS, 16], [1, BS]]),
                          s, C2bT[s], h2b_out, f"1{s}")

            def front(s, t):
                """dec + adds + tanh + scores, self-contained (prologue)."""
                e_dec(s, t)
                e_decT(s, 0)
                for k in range(4):
                    e_add(s, k)
                    if k % 2 == 1:
                        e_tanh(s, k // 2)
                e_scores(s)

            def slot(sb_, sf, t_back, t_front, do_front):
                """Emit stream sb_'s back-half of step t_back, micro-
                interleaved with stream sf's front-half of step t_front."""
                if do_front:
                    e_dec(sf, t_front)
                e_exp(sb_)
                if do_front:
                    e_decT(sf, 0)
                    e_add(sf, 0)
                    e_add(sf, 1)
                e_sums(sb_)
                e_recip(sb_)
                if do_front:
                    e_tanh(sf, 0)
                e_ctx(sb_)
                e_ctxT(sb_)
                e_g0(sb_, t_back)
                if do_front:
                    e_add(sf, 2)
                e_pw0(sb_)
                if do_front:
                    e_add(sf, 3)
                e_g1(sb_, t_back)
                if do_front:
                    e_tanh(sf, 1)
                e_pw1(sb_, t_back)
                if do_front:
                    e_scores(sf)

            # ---- software-pipelined schedule: stream 1 lags half a step --
            front(0, 0)
            for t in range(n_steps):
                slot(0, 1, t, t, True)                       # s0 back | s1 front
                slot(1, 0, t, t + 1, t + 1 < n_steps)        # s1 back | s0 front

            for k in range(4):
                nc.sync.dma_start(out=hballT[k], in_=hball[:, k])

    nc.compile()
    return nc


# --------------------------------------------------------------------------
# Phase B module: logitsT = (0.5*W_out)^T @ H2b_all + b_out  (vocab-sharded)
# --------------------------------------------------------------------------

def build_phase_b():
    nc = bacc.Bacc("TRN2", num_devices=NC, debug=False)
    hbT = nc.dram_tensor("hbT", [4, 128, BT], BF16, kind="ExternalInput").ap()
    wout = nc.dram_tensor("wout", [4, 128, VSL], BF16,
                          kind="ExternalInput").ap()
    boutT = nc.dram_tensor("boutT", [128, 30], F32,
                           kind="ExternalInput").ap()
    logits = nc.dram_tensor("logits", [VSL, BT], BF16,
                            kind="ExternalOutput").ap()

    vtiles = [(vt, vt * 128, min(128, VSL - vt * 128)) for vt in range(30)]
    nchunks = [(0, 512), (512, 512), (1024, 256)]

    with tile.TileContext(nc) as tc:
        with (
            tc.tile_pool(name="w", bufs=1) as wp,
            tc.tile_pool(name="l", bufs=4) as lp,
            tc.tile_pool(name="ps", bufs=2, space="PSUM") as ps,
        ):
            sb_wout = wp.tile([128, 4, VSL], BF16, tag="wout")
            for k in range(4):
                nc.sync.dma_start(out=sb_wout[:, k, 0:128],
                                  in_=wout[k, :, 0:128])
            sb_hbT = wp.tile([128, 4, BT], BF16, tag="hbT")
            for k in range(4):
                for h in range(2):
                    nc.sync.dma_start(out=sb_hbT[:, k, ts(h, BT // 2)],
                                      in_=hbT[k, :, ts(h, BT // 2)])
            sb_boutT = wp.tile([128, 30], F32, tag="boutT")
            nc.sync.dma_start(out=sb_boutT[:], in_=boutT)
            vg = [(128, 832), (960, 960), (1920, 960), (2880, 870)]
            for g0, gw in vg:
                for k in range(4):
                    nc.sync.dma_start(out=sb_wout[:, k, g0:g0 + gw],
                                      in_=wout[k, :, g0:g0 + gw])

            for vt, v0, vw in vtiles:
                pt = ps.tile([128, BT], F32, tag="acc")
                for n0, nw in nchunks:
                    for k in range(4):
                        nc.tensor.matmul(pt[:vw, n0:n0 + nw],
                                         sb_wout[:, k, v0:v0 + vw],
                                         sb_hbT[:, k, n0:n0 + nw],
                                         start=(k == 0), stop=(k == 3),
                                         skip_group_check=True)
                ot = lp.tile([128, BT], BF16, tag="out")
                if vt % 2 == 0:
                    nc.scalar.activation(out=ot[:vw], in_=pt[:vw],
                                         func=AF.Identity,
                                         bias=sb_boutT[:vw, vt:vt + 1])
                else:
                    nc.vector.tensor_scalar(
                        out=ot[:vw], in0=pt[:vw],
                        scalar1=sb_boutT[:vw, vt:vt + 1], scalar2=None,
                        op0=ALU.add)
                nc.sync.dma_start(out=logits[v0:v0 + vw], in_=ot[:vw])
    nc.compile()
    return nc


# --------------------------------------------------------------------------
# Host-side preparation + driver
# --------------------------------------------------------------------------

def prep_phase_a_inputs(features, encoder_out, emb, W_enc, b_enc, W_dec, b_dec,
                        v_w, W_g0, b_g0, W_g1, b_g1, W_ih, b_ih, W_ic, b_ic,
                        captions):
    embeds = np.asarray(emb)[np.asarray(captions)[:, :T].astype(np.int64)]
    cs = np.ones((G4,), np.float32)     # sigmoid halving on f, i, o columns
    cs[0:H] = 0.5
    cs[H:2 * H] = 0.5
    cs[3 * H:4 * H] = 0.5
    W_g0 = np.asarray(W_g0) * cs
    W_g1 = np.asarray(W_g1) * cs
    shared = {
        "wih2": _bf16(2.0 * np.asarray(W_ih).reshape(4, 128, H)),
        "wic2": _bf16(2.0 * np.asarray(W_ic).reshape(4, 128, H)),
        "bih2": _bf16(2.0 * np.asarray(b_ih).reshape(1, H)),
        "bic2": _bf16(2.0 * np.asarray(b_ic).reshape(1, H)),
        "wenc": _bf16(np.asarray(W_enc).reshape(4, 128, A)),
        "biasadT": _bf16((np.asarray(b_enc) + np.asarray(b_dec))
                         .reshape(4, 128, 1)),
        "wdech": _bf16(0.5 * np.asarray(W_dec).reshape(4, 128, A)),
        "vcol": _bf16(np.asarray(v_w).reshape(4, 128, 1)),
        "wg0x": _bf16(W_g0[:E].reshape(4, 128, G4)),
        "bg0": _bf16((np.asarray(b_g0) * cs).reshape(1, G4)),
        "wg0c": _bf16(W_g0[E:2 * E].reshape(4, 128, G4)),
        "wg0h": _bf16(0.5 * W_g0[2 * E:].reshape(4, 128, G4)),
        "wg1a": _bf16(0.5 * W_g1[:H].reshape(4, 128, G4)),
        "wg1b": _bf16(0.5 * W_g1[H:].reshape(4, 128, G4)),
        "bg1": _bf16((np.asarray(b_g1) * cs).reshape(1, G4)),
    }
    in_maps = []
    for c in range(NC):
        bs = slice(c * BL, (c + 1) * BL)
        enc = np.asarray(encoder_out)[bs]               # [8, 196, 512]
        encTn = enc.transpose(2, 1, 0).reshape(E, BP)   # (e, p, b)
        encpad = np.zeros((BL, PPAD, E), np.float32)
        encpad[:, :P] = enc
        feat = np.asarray(features)[bs]
        we = embeds[bs]                                 # [8, T, E]
        m = dict(shared)
        m["encT"] = _bf16(encTn.reshape(4, 128, BP))
        m["encflat"] = _bf16(encpad.reshape(NF, 128, E))
        m["featT"] = _bf16(feat.T.reshape(4, 128, BL))
        m["weT"] = _bf16(we.transpose(2, 1, 0).reshape(4, 128, T * BL))
        in_maps.append(m)
    return in_maps


_CACHE = {}


def kernel(**inputs):
    inputs = {k: np.asarray(v) for k, v in inputs.items()}
    if "a" not in _CACHE:
        _CACHE["a"] = build_phase_a()
    if "b" not in _CACHE:
        _CACHE["b"] = build_phase_b()

    in_a = prep_phase_a_inputs(
        inputs["features"], inputs["encoder_out"], inputs["emb"],
        inputs["W_enc"], inputs["b_enc"], inputs["W_dec"], inputs["b_dec"],
        inputs["v_w"], inputs["W_g0"], inputs["b_g0"], inputs["W_g1"],
        inputs["b_g1"], inputs["W_ih"], inputs["b_ih"], inputs["W_ic"],
        inputs["b_ic"], inputs["captions"])
    ra = run_bass_kernel_spmd(_CACHE["a"], in_a, core_ids=list(range(NC)))

    # reassemble hb: column index b*T + t
    hbT_full = np.zeros((4, 128, BT), dtype=ml_dtypes.bfloat16)
    for c in range(NC):
        part = ra.results[c]["hballT"].reshape(4, 128, T, BL)
        for bl in range(BL):
            b = c * BL + bl
            hbT_full[:, :, b * T:(b + 1) * T] = part[:, :, :, bl]

    W_out = np.asarray(inputs["W_out"])
    b_out = np.asarray(inputs["b_out"])
    bpad = np.zeros((30 * 128,), np.float32)
    in_b = []
    for c in range(NC):
        vs = slice(c * VSL, (c + 1) * VSL)
        bpad[:VSL] = b_out[vs]
        in_b.append({
            "hbT": hbT_full,
            "wout": _bf16(0.5 * W_out[:, vs].reshape(4, 128, VSL)),
            "boutT": np.ascontiguousarray(bpad.reshape(30, 128).T,
                                          dtype=np.float32),
        })
    rb = run_bass_kernel_spmd(_CACHE["b"], in_b, core_ids=list(range(NC)))
    out = np.empty((BT, V), np.float32)
    for c in range(NC):
        vs = slice(c * VSL, (c + 1) * VSL)
        out[:, vs] = rb.results[c]["logits"].astype(np.float32).T
    return out.reshape(B, T, V)
